# revision 10
# baseline (speedup 1.0000x reference)
"""TimeSformer divided-space attention (nn_Attention_3856880632239) on 8 TRN2 cores.

Sharding: core c = (batch-group bg = c//4) x (head-pair hg = c%4).
Each core handles 2 batches x 2 heads:
  - qkv projection in transposed layout (feature-on-partition), fp32r matmuls
  - per (b,h,frame): scoresT = k @ q^T (keys on partitions), exp (bf16),
    AV in natural orientation (queries on partitions) so softmax
    normalization is a per-partition scalar multiply, then PE-transpose back.
  - CLS query handled via query replication per frame section + partial-sum
    accumulation; CLS key handled via a batched rank-1 (K=1) matmul.
  - out projection with W_out row slice; partials summed on host.
"""
import numpy as np
from contextlib import ExitStack

import concourse.bass as bass
from concourse import bacc
import concourse.tile as tile
import concourse.mybir as mybir
from concourse import bass_utils
from concourse.masks import make_identity

F32 = mybir.dt.float32
F32R = mybir.dt.float32r
BF16 = mybir.dt.bfloat16
AF = mybir.ActivationFunctionType
ALU = mybir.AluOpType

B, N, D, H, DH, NF, NP = 4, 3137, 512, 8, 64, 16, 196
NB = 2            # batches per core
SEC = NP + 1      # 197: section = [cls | frame queries]
QCOLS = 3584      # sectioned query buffer (16*197=3152, padded to 7*512)
KCOLS = N         # global key/row space
SCALE = DH ** -0.5


def _sec_pieces(g0, g1):
    """Global row range [g0,g1) (g0>=1) -> (src_off, dst_col, len) pieces."""
    out = []
    g = g0
    while g < g1:
        s, j = (g - 1) // NP, (g - 1) % NP
        run = min(g1 - g, NP - j)
        out.append((g - g0, SEC * s + 1 + j, run))
        g += run
    return out


def build():
    nc = bacc.Bacc("TRN2", target_bir_lowering=False, debug=False, num_devices=8)
    x = nc.dram_tensor("x", [NB * N, D], F32, kind="ExternalInput")
    wq = nc.dram_tensor("wq", [D, 128], F32, kind="ExternalInput")
    wk = nc.dram_tensor("wk", [D, 128], F32, kind="ExternalInput")
    wv = nc.dram_tensor("wv", [D, 128], F32, kind="ExternalInput")
    wo = nc.dram_tensor("wo", [128, D], F32, kind="ExternalInput")
    y = nc.dram_tensor("y", [NB * N, D], F32, kind="ExternalOutput")

    with tile.TileContext(nc) as tc, ExitStack() as ctx:
        consts = ctx.enter_context(tc.tile_pool(name="consts", bufs=1))
        wrk = ctx.enter_context(tc.tile_pool(name="wrk", bufs=3))
        xtp = ctx.enter_context(tc.tile_pool(name="xtp", bufs=2))
        smallp = ctx.enter_context(tc.tile_pool(name="smallp", bufs=2))
        ps_big = ctx.enter_context(tc.tile_pool(name="ps_big", bufs=3, space="PSUM"))
        ps_avt = ctx.enter_context(tc.tile_pool(name="ps_avt", bufs=3, space="PSUM"))
        ps_at = ctx.enter_context(tc.tile_pool(name="ps_at", bufs=2, space="PSUM"))

        ident_f = consts.tile([128, 128], F32, tag="identf")
        make_identity(nc, ident_f[:])
        ident_b = consts.tile([128, 128], BF16, tag="identb")
        make_identity(nc, ident_b[:])

        # weights -> SBUF, rounded to f32r
        wq_f = consts.tile([128, 4, 128], F32, tag="wqf")
        wk_f = consts.tile([128, 4, 128], F32, tag="wkf")
        wv_f = consts.tile([128, 4, 128], F32, tag="wvf")
        for wt, wdr in ((wq_f, wq), (wk_f, wk), (wv_f, wv)):
            nc.sync.dma_start(wt[:], wdr[:].rearrange("(k p) c -> p k c", p=128))
        wq_r = consts.tile([128, 4, 128], F32R, tag="wqr")
        wk_r = consts.tile([128, 4, 128], F32R, tag="wkr")
        wv_r = consts.tile([128, 4, 128], F32R, tag="wvr")
        nc.scalar.copy(wq_r[:], wq_f[:])
        nc.scalar.copy(wk_r[:], wk_f[:])
        nc.scalar.copy(wv_r[:], wv_f[:])
        wo_f = consts.tile([128, D], F32, tag="wof")
        nc.sync.dma_start(wo_f[:], wo[:])
        wo_r = consts.tile([128, D], F32R, tag="wor")
        nc.scalar.copy(wo_r[:], wo_f[:])

        qT, kT, vT, aT = [], [], [], []
        for b in range(NB):
            qT.append(consts.tile([128, QCOLS], F32R, tag=f"qT{b}", name=f"qT{b}"))
            kT.append(consts.tile([128, KCOLS], F32R, tag=f"kT{b}", name=f"kT{b}"))
            vT.append(consts.tile([128, KCOLS], BF16, tag=f"vT{b}", name=f"vT{b}"))
            aT.append(consts.tile([128, KCOLS], F32R, tag=f"aT{b}", name=f"aT{b}"))
        zpad = consts.tile([128, QCOLS - 16 * SEC], F32, tag="zpad")
        nc.vector.memset(zpad[:], 0.0)
        for b in range(NB):
            nc.scalar.copy(qT[b][:, 16 * SEC:QCOLS], zpad[:])

        copy_flip = [0]

        def copy_alt(dst, src):
            copy_flip[0] ^= 1
            if copy_flip[0]:
                nc.scalar.copy(dst, src)
            else:
                nc.vector.tensor_copy(dst, src)

        # ---------------- Phase A: transpose x + qkv projection ------------
        def a_sb(b, r0):
                nsb = min(512, N - r0)
                xtt = xtp.tile([128, 4, 512], F32R, tag="xT")
                xa = wrk.tile([128, 4, 512], F32, tag="xa")
                nfull = nsb // 128
                if nfull:
                    nc.sync.dma_start(
                        xa[:, 0:nfull, :],
                        x[b * N + r0: b * N + r0 + nfull * 128, :]
                        .rearrange("(c p) d -> p c d", p=128))
                if nsb % 128:
                    nc.sync.dma_start(
                        xa[0:nsb % 128, nfull, :],
                        x[b * N + r0 + nfull * 128: b * N + r0 + nsb, :])
                for rr0 in range(r0, r0 + nsb, 128):
                    nr = min(128, N - rr0)
                    c = (rr0 - r0) // 128
                    pack = ps_big.tile([128, 512], F32, tag="big")
                    for k in range(4):
                        nc.tensor.transpose(
                            pack[:, k * 128: k * 128 + nr],
                            xa[0:nr, c, k * 128:(k + 1) * 128],
                            ident_f[0:nr, 0:nr])
                    off = rr0 - r0
                    copy_alt(
                        xtt[:, :, off:off + nr],
                        pack[:].rearrange("p (k c) -> p k c", k=4)[:, :, 0:nr])
                nmm = nsb + (nsb & 1)  # fp32r needs an even moving dim
                for wt, g in ((wq_r, 0), (wk_r, 1), (wv_r, 2)):
                    pp = ps_big.tile([128, 512], F32, tag="big")
                    for k in range(4):
                        nc.tensor.matmul(pp[:, 0:nmm], wt[:, k, :],
                                         xtt[:, k, 0:nmm],
                                         start=(k == 0), stop=(k == 3))
                    if g == 0:
                        if r0 == 0:
                            for s in range(NF):
                                nc.scalar.copy(qT[b][:, SEC * s: SEC * s + 1],
                                               pp[:, 0:1])
                        g0 = max(r0, 1)
                        for src, dstc, ln in _sec_pieces(g0, r0 + nsb):
                            nc.scalar.copy(qT[b][:, dstc:dstc + ln],
                                           pp[:, (g0 - r0) + src:(g0 - r0) + src + ln])
                    elif g == 1:
                        nc.vector.tensor_copy(kT[b][:, r0:r0 + nsb], pp[:, 0:nsb])
                    else:
                        nc.vector.tensor_copy(vT[b][:, r0:r0 + nsb], pp[:, 0:nsb])

        def phase_a(b):
            for r0 in range(0, N, 512):
                a_sb(b, r0)

        # ---------------- Phase S: batched CLS-key scores ------------------
        def phase_s(b, h, e_cls):
            pb = h * 64
            for c0 in range(0, QCOLS, 512):
                sp = ps_big.tile([1, 512], F32, tag="big")
                nc.tensor.matmul(sp[0:1, :], kT[b][pb:pb + 64, 0:1],
                                 qT[b][pb:pb + 64, c0:c0 + 512],
                                 start=True, stop=True)
                nc.scalar.activation(e_cls[0:1, c0:c0 + 512], sp[0:1, :],
                                     AF.Exp, scale=SCALE)

        # ---------------- Phase B: per-frame attention ---------------------
        def block(b, h, f, e_cls, cv, clsacc):
            pb = h * 64
            w0 = SEC * f
            kc = 1 + NP * f
            # scoresT: [keys, window-queries]
            sc = ps_big.tile([128, 512], F32, tag="big")
            nc.tensor.matmul(sc[0:128, 0:256], kT[b][pb:pb + 64, kc:kc + 128],
                             qT[b][pb:pb + 64, w0:w0 + 256], start=True, stop=True)
            nc.tensor.matmul(sc[0:68, 256:512], kT[b][pb:pb + 64, kc + 128:kc + 196],
                             qT[b][pb:pb + 64, w0:w0 + 256], start=True, stop=True)
            expT = wrk.tile([128, 394], BF16, tag="expT")
            nc.scalar.activation(expT[0:128, 0:197], sc[0:128, 0:197],
                                 AF.Exp, scale=SCALE)
            nc.scalar.activation(expT[0:68, 197:394], sc[0:68, 256:453],
                                 AF.Exp, scale=SCALE)
            # v natural (+ ones col) via PE transpose of vT
            vp = ps_avt.tile([128, 128], BF16, tag="avt")
            nc.tensor.transpose(vp[0:128, 0:64], vT[b][pb:pb + 64, kc:kc + 128],
                                ident_b[pb:pb + 64, pb:pb + 64])
            nc.tensor.transpose(vp[0:68, 64:128], vT[b][pb:pb + 64, kc + 128:kc + 196],
                                ident_b[pb:pb + 64, pb:pb + 64])
            ve = wrk.tile([128, 130], BF16, tag="ve")
            nc.vector.tensor_copy(
                ve[0:128, :].rearrange("p (g c) -> p g c", g=2)[:, :, 0:64],
                vp[0:128, :].rearrange("p (g c) -> p g c", g=2))
            nc.gpsimd.memset(ve[0:128, 64:65], 1.0)
            nc.gpsimd.memset(ve[0:68, 129:130], 1.0)
            # AV in natural orientation; col 64/129 = sumexp
            avp = ps_avt.tile([128, 130], F32, tag="avt")
            nc.tensor.matmul(avp[0:128, 0:65], expT[0:128, 0:128], ve[0:128, 0:65],
                             start=True, stop=False)
            nc.tensor.matmul(avp[0:128, 0:65], expT[0:68, 197:325], ve[0:68, 65:130],
                             start=False, stop=False)
            nc.tensor.matmul(avp[0:128, 0:65], e_cls[0:1, w0:w0 + 128], cv[0:1, 0:65],
                             start=False, stop=True)
            nc.tensor.matmul(avp[0:69, 65:130], expT[0:128, 128:197], ve[0:128, 0:65],
                             start=True, stop=False)
            nc.tensor.matmul(avp[0:69, 65:130], expT[0:68, 325:394], ve[0:68, 65:130],
                             start=False, stop=False)
            nc.tensor.matmul(avp[0:69, 65:130], e_cls[0:1, w0 + 128:w0 + 197],
                             cv[0:1, 0:65], start=False, stop=True)
            # cls partial accumulation (row 0 = cls query)
            nc.vector.tensor_copy(clsacc[0:1, 65 * f:65 * f + 65], avp[0:1, 0:65])
            # normalize
            rinv = smallp.tile([128, 2], F32, tag="rinv")
            nc.vector.reciprocal(
                rinv[:], avp[0:128, :].rearrange("p (g c) -> p g c", g=2)[:, :, 64])
            ana = wrk.tile([128, 64], BF16, tag="ana")
            anb = wrk.tile([69, 64], BF16, tag="anb")
            nc.vector.tensor_scalar_mul(ana[:], avp[0:128, 0:64], rinv[:, 0:1])
            nc.vector.tensor_scalar_mul(anb[:], avp[0:69, 65:129], rinv[0:69, 1:2])
            atp = ps_at.tile([64, 197], BF16, tag="at")
            nc.tensor.transpose(atp[0:64, 0:128], ana[:], ident_b[:])
            nc.tensor.transpose(atp[0:64, 128:197], anb[:], ident_b[0:69, 0:69])
            nc.vector.tensor_copy(aT[b][pb:pb + 64, kc:kc + NP], atp[0:64, 1:197])

        def cls_final(b, h, e_cls, cv, clsacc):
            pb = h * 64
            clssum = smallp.tile([1, 65], F32, tag="clssum")
            nc.vector.reduce_sum(
                clssum[:], clsacc[0:1, :].rearrange("p (s c) -> p c s", s=NF),
                axis=mybir.AxisListType.X)
            corr = smallp.tile([1, 65], F32, tag="corr")
            eself = smallp.tile([1, 1], F32, tag="eself")
            nc.vector.tensor_copy(eself[:], e_cls[0:1, 0:1])
            nc.vector.tensor_scalar(corr[:], cv[0:1, 0:65], eself[:],
                                    float(NF - 1), op0=ALU.mult, op1=ALU.mult)
            nc.vector.tensor_tensor(out=clssum[:], in0=clssum[:], in1=corr[:],
                                    op=ALU.subtract)
            crinv = smallp.tile([1, 1], F32, tag="crinv")
            nc.vector.reciprocal(crinv[:], clssum[0:1, 64:65])
            clsa = smallp.tile([1, 64], F32, tag="clsa")
            nc.vector.tensor_scalar_mul(clsa[:], clssum[0:1, 0:64], crinv[:])
            ctp = ps_avt.tile([64, 197], F32, tag="avt")
            nc.tensor.transpose(ctp[0:64, 0:1], clsa[:], ident_f[0:1, 0:1])
            nc.vector.tensor_copy(aT[b][pb:pb + 64, 0:1], ctp[0:64, 0:1])

        # ---------------- Phase C: out projection --------------------------
        def c_sb(b, s0):
                nsb = min(512, N - s0)
                ysb = wrk.tile([128, 4, 512], F32, tag="ysb")
                for r0 in range(s0, s0 + nsb, 128):
                    nr = min(128, N - r0)
                    c = (r0 - s0) // 128
                    yp = ps_big.tile([128, 512], F32, tag="big")
                    nc.tensor.matmul(yp[0:nr, :], aT[b][:, r0:r0 + nr], wo_r[:],
                                     start=True, stop=True)
                    copy_alt(ysb[0:nr, c, :], yp[0:nr, :])
                nfull = nsb // 128
                if nfull:
                    nc.sync.dma_start(
                        y[b * N + s0: b * N + s0 + nfull * 128, :]
                        .rearrange("(c p) d -> p c d", p=128),
                        ysb[:, 0:nfull, :])
                if nsb % 128:
                    nc.sync.dma_start(
                        y[b * N + s0 + nfull * 128: b * N + s0 + nsb, :],
                        ysb[0:nsb % 128, nfull, :])

        def b_units(b):
            units = []
            for h in range(2):
                state = {}

                def setup(b=b, h=h, state=state):
                    e_cls = smallp.tile([1, QCOLS], BF16, tag="ecls", name="e_cls")
                    cv = smallp.tile([1, 65], BF16, tag="cv", name="cv")
                    clsacc = smallp.tile([1, 65 * NF], F32, tag="clsacc",
                                         name="clsacc")
                    pb = h * 64
                    cvp = ps_at.tile([64, 197], BF16, tag="at", name="cvp")
                    nc.tensor.transpose(cvp[0:1, 0:64], vT[b][pb:pb + 64, 0:1],
                                        ident_b[pb:pb + 64, pb:pb + 64])
                    nc.vector.tensor_copy(cv[0:1, 0:64], cvp[0:1, 0:64])
                    nc.vector.memset(cv[0:1, 64:65], 1.0)
                    phase_s(b, h, e_cls)
                    state.update(e_cls=e_cls, cv=cv, clsacc=clsacc)
                units.append(setup)
                for f in range(NF):
                    units.append(lambda b=b, h=h, f=f, state=state: block(
                        b, h, f, state["e_cls"], state["cv"], state["clsacc"]))
                units.append(lambda b=b, h=h, state=state: cls_final(
                    b, h, state["e_cls"], state["cv"], state["clsacc"]))
            return units

        def interleave(main, aux, period):
            ai = 0
            for i, u in enumerate(main):
                u()
                if (i + 1) % period == 0 and ai < len(aux):
                    aux[ai]()
                    ai += 1
            while ai < len(aux):
                aux[ai]()
                ai += 1

        a_units_1 = [lambda r0=r0: a_sb(1, r0) for r0 in range(0, N, 512)]
        c_units_0 = [lambda s0=s0: c_sb(0, s0) for s0 in range(0, N, 512)]
        c_units_1 = [lambda s0=s0: c_sb(1, s0) for s0 in range(0, N, 512)]

        phase_a(0)
        interleave(b_units(0), a_units_1, 5)
        interleave(b_units(1), c_units_0, 5)
        for u in c_units_1:
            u()
    nc.finalize()
    return nc


_NC_CACHE = None


def _get_nc():
    global _NC_CACHE
    if _NC_CACHE is None:
        _NC_CACHE = build()
    return _NC_CACHE


def make_in_maps(x, W_qkv, W_out, b_out):
    x = np.asarray(x, dtype=np.float32)
    W_qkv = np.asarray(W_qkv, dtype=np.float32)
    W_out = np.asarray(W_out, dtype=np.float32)
    b_out = np.asarray(b_out, dtype=np.float32)
    in_maps = []
    for c in range(8):
        bg, hg = c // 4, c % 4
        cs = hg * 128
        m = {
            "x": np.ascontiguousarray(
                x[2 * bg:2 * bg + 2].reshape(NB * N, D)),
            "wq": np.ascontiguousarray(W_qkv[:, cs:cs + 128]),
            "wk": np.ascontiguousarray(W_qkv[:, 512 + cs:512 + cs + 128]),
            "wv": np.ascontiguousarray(W_qkv[:, 1024 + cs:1024 + cs + 128]),
            "wo": np.ascontiguousarray(W_out[cs:cs + 128, :]),
        }
        in_maps.append(m)
    return in_maps


def assemble(results, b_out):
    y = np.empty((B, N, D), dtype=np.float32)
    for bg in range(2):
        acc = results[bg * 4]["y"].copy()
        for hg in range(1, 4):
            acc += results[bg * 4 + hg]["y"]
        acc += np.asarray(b_out, dtype=np.float32)[None, :]
        y[2 * bg:2 * bg + 2] = acc.reshape(NB, N, D)
    return y


def kernel(x, W_qkv, W_out, b_out, f):
    assert int(f) == NF
    nc = _get_nc()
    in_maps = make_in_maps(x, W_qkv, W_out, b_out)
    res = bass_utils.run_bass_kernel_spmd(nc, in_maps, core_ids=list(range(8)))
    return assemble(res.results, b_out)


# revision 11
# speedup vs baseline: 1.1362x; 1.1362x over previous
"""TimeSformer divided-space attention (nn_Attention_3856880632239) on 8 TRN2 cores.

Sharding: core c = (batch-group bg = c//4) x (head-pair hg = c%4).
Each core handles 2 batches x 2 heads:
  - qkv projection in transposed layout (feature-on-partition), fp32r matmuls
  - per (b,h,frame): scoresT = k @ q^T (keys on partitions), exp (bf16),
    AV in natural orientation (queries on partitions) so softmax
    normalization is a per-partition scalar multiply, then PE-transpose back.
  - CLS query handled via query replication per frame section + partial-sum
    accumulation; CLS key handled via a batched rank-1 (K=1) matmul.
  - out projection with W_out row slice; partials summed on host.
"""
import numpy as np
from contextlib import ExitStack

import concourse.bass as bass
from concourse import bacc
import concourse.tile as tile
import concourse.mybir as mybir
from concourse import bass_utils
from concourse.masks import make_identity

F32 = mybir.dt.float32
F32R = mybir.dt.float32r
BF16 = mybir.dt.bfloat16
AF = mybir.ActivationFunctionType
ALU = mybir.AluOpType

B, N, D, H, DH, NF, NP = 4, 3137, 512, 8, 64, 16, 196
NB = 2            # batches per core
SEC = NP + 1      # 197: section = [cls | frame queries]
QCOLS = 3584      # sectioned query buffer (16*197=3152, padded to 7*512)
KCOLS = N         # global key/row space
SCALE = DH ** -0.5


def _sec_pieces(g0, g1):
    """Global row range [g0,g1) (g0>=1) -> (src_off, dst_col, len) pieces."""
    out = []
    g = g0
    while g < g1:
        s, j = (g - 1) // NP, (g - 1) % NP
        run = min(g1 - g, NP - j)
        out.append((g - g0, SEC * s + 1 + j, run))
        g += run
    return out


def build():
    nc = bacc.Bacc("TRN2", target_bir_lowering=False, debug=False, num_devices=8)
    x = nc.dram_tensor("x", [NB * N, D], F32, kind="ExternalInput")
    wq = nc.dram_tensor("wq", [D, 128], F32, kind="ExternalInput")
    wk = nc.dram_tensor("wk", [D, 128], F32, kind="ExternalInput")
    wv = nc.dram_tensor("wv", [D, 128], F32, kind="ExternalInput")
    wo = nc.dram_tensor("wo", [128, D], F32, kind="ExternalInput")
    y = nc.dram_tensor("y", [NB * N, D], F32, kind="ExternalOutput")

    with tile.TileContext(nc) as tc, ExitStack() as ctx:
        consts = ctx.enter_context(tc.tile_pool(name="consts", bufs=1))
        wrk = ctx.enter_context(tc.tile_pool(name="wrk", bufs=3))
        xtp = ctx.enter_context(tc.tile_pool(name="xtp", bufs=2))
        smallp = ctx.enter_context(tc.tile_pool(name="smallp", bufs=2))
        ps_big = ctx.enter_context(tc.tile_pool(name="ps_big", bufs=4, space="PSUM"))
        ps_avt = ctx.enter_context(tc.tile_pool(name="ps_avt", bufs=2, space="PSUM"))
        ps_at = ctx.enter_context(tc.tile_pool(name="ps_at", bufs=2, space="PSUM"))

        ident_f = consts.tile([128, 128], F32, tag="identf")
        make_identity(nc, ident_f[:])
        ident_b = consts.tile([128, 128], BF16, tag="identb")
        make_identity(nc, ident_b[:])

        # weights -> SBUF, rounded to f32r
        wq_f = consts.tile([128, 4, 128], F32, tag="wqf")
        wk_f = consts.tile([128, 4, 128], F32, tag="wkf")
        wv_f = consts.tile([128, 4, 128], F32, tag="wvf")
        for wt, wdr in ((wq_f, wq), (wk_f, wk), (wv_f, wv)):
            nc.sync.dma_start(wt[:], wdr[:].rearrange("(k p) c -> p k c", p=128))
        wq_r = consts.tile([128, 4, 128], F32R, tag="wqr")
        wk_r = consts.tile([128, 4, 128], F32R, tag="wkr")
        wv_r = consts.tile([128, 4, 128], F32R, tag="wvr")
        nc.scalar.copy(wq_r[:], wq_f[:])
        nc.scalar.copy(wk_r[:], wk_f[:])
        nc.scalar.copy(wv_r[:], wv_f[:])
        wo_f = consts.tile([128, D], F32, tag="wof")
        nc.sync.dma_start(wo_f[:], wo[:])
        wo_r = consts.tile([128, D], F32R, tag="wor")
        nc.scalar.copy(wo_r[:], wo_f[:])

        qT, kT, vT, aT = [], [], [], []
        for b in range(NB):
            qT.append(consts.tile([128, QCOLS], F32R, tag=f"qT{b}", name=f"qT{b}"))
            kT.append(consts.tile([128, KCOLS], F32R, tag=f"kT{b}", name=f"kT{b}"))
            vT.append(consts.tile([128, KCOLS], BF16, tag=f"vT{b}", name=f"vT{b}"))
            aT.append(consts.tile([128, KCOLS], F32R, tag=f"aT{b}", name=f"aT{b}"))
        zpad = consts.tile([128, QCOLS - 16 * SEC], F32, tag="zpad")
        nc.vector.memset(zpad[:], 0.0)
        for b in range(NB):
            nc.scalar.copy(qT[b][:, 16 * SEC:QCOLS], zpad[:])

        copy_flip = [0]

        def copy_alt(dst, src):
            copy_flip[0] ^= 1
            if copy_flip[0]:
                nc.scalar.copy(dst, src)
            else:
                nc.vector.tensor_copy(dst, src)

        # ---------------- Phase A: transpose x + qkv projection ------------
        def a_sb(b, r0):
                nsb = min(512, N - r0)
                xtt = xtp.tile([128, 4, 512], F32R, tag="xT")
                xa = wrk.tile([128, 4, 512], F32, tag="xa")
                nfull = nsb // 128
                if nfull:
                    nc.sync.dma_start(
                        xa[:, 0:nfull, :],
                        x[b * N + r0: b * N + r0 + nfull * 128, :]
                        .rearrange("(c p) d -> p c d", p=128))
                if nsb % 128:
                    nc.sync.dma_start(
                        xa[0:nsb % 128, nfull, :],
                        x[b * N + r0 + nfull * 128: b * N + r0 + nsb, :])
                for rr0 in range(r0, r0 + nsb, 128):
                    nr = min(128, N - rr0)
                    c = (rr0 - r0) // 128
                    pack = ps_big.tile([128, 512], F32, tag="big")
                    for k in range(4):
                        nc.tensor.transpose(
                            pack[:, k * 128: k * 128 + nr],
                            xa[0:nr, c, k * 128:(k + 1) * 128],
                            ident_f[0:nr, 0:nr])
                    off = rr0 - r0
                    copy_alt(
                        xtt[:, :, off:off + nr],
                        pack[:].rearrange("p (k c) -> p k c", k=4)[:, :, 0:nr])
                nmm = nsb + (nsb & 1)  # fp32r needs an even moving dim
                for wt, g in ((wq_r, 0), (wk_r, 1), (wv_r, 2)):
                    pp = ps_big.tile([128, 512], F32, tag="big")
                    for k in range(4):
                        nc.tensor.matmul(pp[:, 0:nmm], wt[:, k, :],
                                         xtt[:, k, 0:nmm],
                                         start=(k == 0), stop=(k == 3))
                    if g == 0:
                        if r0 == 0:
                            for s in range(NF):
                                nc.scalar.copy(qT[b][:, SEC * s: SEC * s + 1],
                                               pp[:, 0:1])
                        g0 = max(r0, 1)
                        for src, dstc, ln in _sec_pieces(g0, r0 + nsb):
                            nc.scalar.copy(qT[b][:, dstc:dstc + ln],
                                           pp[:, (g0 - r0) + src:(g0 - r0) + src + ln])
                    elif g == 1:
                        nc.vector.tensor_copy(kT[b][:, r0:r0 + nsb], pp[:, 0:nsb])
                    else:
                        nc.vector.tensor_copy(vT[b][:, r0:r0 + nsb], pp[:, 0:nsb])

        def phase_a(b):
            for r0 in range(0, N, 512):
                a_sb(b, r0)

        # ---------------- Phase S: batched CLS-key scores ------------------
        def phase_s(b, h, e_cls):
            pb = h * 64
            for c0 in range(0, QCOLS, 512):
                sp = ps_big.tile([1, 512], F32, tag="big")
                nc.tensor.matmul(sp[0:1, :], kT[b][pb:pb + 64, 0:1],
                                 qT[b][pb:pb + 64, c0:c0 + 512],
                                 start=True, stop=True)
                nc.scalar.activation(e_cls[0:1, c0:c0 + 512], sp[0:1, :],
                                     AF.Exp, scale=SCALE)

        # ---------------- Phase B: per-frame attention ---------------------
        def ve_prebuild(b, h):
            pb = h * 64
            veb = wrk.tile([128, NF * 130], BF16, tag="veb", name="veb", bufs=2)
            for f in range(NF):
                kc = 1 + NP * f
                vp = ps_avt.tile([128, 128], BF16, tag="avt", name="vp")
                nc.tensor.transpose(vp[0:128, 0:64], vT[b][pb:pb + 64, kc:kc + 128],
                                    ident_b[pb:pb + 64, pb:pb + 64])
                nc.tensor.transpose(vp[0:68, 64:128],
                                    vT[b][pb:pb + 64, kc + 128:kc + 196],
                                    ident_b[pb:pb + 64, pb:pb + 64])
                o = 130 * f
                nc.vector.tensor_copy(
                    veb[0:128, o:o + 130].rearrange(
                        "p (g c) -> p g c", g=2)[:, :, 0:64],
                    vp[0:128, :].rearrange("p (g c) -> p g c", g=2))
            nc.gpsimd.memset(
                veb[0:128, :].rearrange("p (s c) -> p s c", s=NF)[:, :, 64:65], 1.0)
            nc.gpsimd.memset(
                veb[0:68, :].rearrange("p (s c) -> p s c", s=NF)[:, :, 129:130], 1.0)
            return veb

        def b_stage1(b, h, f, st):
            pb = h * 64
            w0 = SEC * f
            kc = 1 + NP * f
            # scoresT: [keys, window-queries]
            sc = ps_big.tile([128, 512], F32, tag="big", name="sc")
            nc.tensor.matmul(sc[0:128, 0:256], kT[b][pb:pb + 64, kc:kc + 128],
                             qT[b][pb:pb + 64, w0:w0 + 256], start=True, stop=True)
            nc.tensor.matmul(sc[0:68, 256:512], kT[b][pb:pb + 64, kc + 128:kc + 196],
                             qT[b][pb:pb + 64, w0:w0 + 256], start=True, stop=True)
            expT = wrk.tile([128, 394], BF16, tag="expT", name="expT")
            nc.scalar.activation(expT[0:128, 0:197], sc[0:128, 0:197],
                                 AF.Exp, scale=SCALE)
            nc.scalar.activation(expT[0:68, 197:394], sc[0:68, 256:453],
                                 AF.Exp, scale=SCALE)
            st["expT"] = expT

        def b_stage2(b, h, f, st):
            pb = h * 64
            w0 = SEC * f
            kc = 1 + NP * f
            e_cls, cv, clsacc = st["e_cls"], st["cv"], st["clsacc"]
            expT, veb = st["expT"], st["veb"]
            ve = veb[:, 130 * f:130 * f + 130]
            avp = ps_avt.tile([128, 130], F32, tag="avt", name="avp")
            nc.tensor.matmul(avp[0:128, 0:65], expT[0:128, 0:128], ve[0:128, 0:65],
                             start=True, stop=False)
            nc.tensor.matmul(avp[0:128, 0:65], expT[0:68, 197:325], ve[0:68, 65:130],
                             start=False, stop=False)
            nc.tensor.matmul(avp[0:128, 0:65], e_cls[0:1, w0:w0 + 128], cv[0:1, 0:65],
                             start=False, stop=True)
            nc.tensor.matmul(avp[0:69, 65:130], expT[0:128, 128:197], ve[0:128, 0:65],
                             start=True, stop=False)
            nc.tensor.matmul(avp[0:69, 65:130], expT[0:68, 325:394], ve[0:68, 65:130],
                             start=False, stop=False)
            nc.tensor.matmul(avp[0:69, 65:130], e_cls[0:1, w0 + 128:w0 + 197],
                             cv[0:1, 0:65], start=False, stop=True)
            # cls partial accumulation (row 0 = cls query)
            nc.vector.tensor_copy(clsacc[0:1, 65 * f:65 * f + 65], avp[0:1, 0:65])
            # normalize
            rinv = smallp.tile([128, 2], F32, tag="rinv")
            nc.vector.reciprocal(
                rinv[:], avp[0:128, :].rearrange("p (g c) -> p g c", g=2)[:, :, 64])
            ana = wrk.tile([128, 64], BF16, tag="ana")
            anb = wrk.tile([69, 64], BF16, tag="anb")
            nc.vector.tensor_scalar_mul(ana[:], avp[0:128, 0:64], rinv[:, 0:1])
            nc.vector.tensor_scalar_mul(anb[:], avp[0:69, 65:129], rinv[0:69, 1:2])
            atp = ps_at.tile([64, 197], BF16, tag="at")
            nc.tensor.transpose(atp[0:64, 0:128], ana[:], ident_b[:])
            nc.tensor.transpose(atp[0:64, 128:197], anb[:], ident_b[0:69, 0:69])
            nc.vector.tensor_copy(aT[b][pb:pb + 64, kc:kc + NP], atp[0:64, 1:197])

        def cls_final(b, h, e_cls, cv, clsacc):
            pb = h * 64
            clssum = smallp.tile([1, 65], F32, tag="clssum")
            nc.vector.reduce_sum(
                clssum[:], clsacc[0:1, :].rearrange("p (s c) -> p c s", s=NF),
                axis=mybir.AxisListType.X)
            corr = smallp.tile([1, 65], F32, tag="corr")
            eself = smallp.tile([1, 1], F32, tag="eself")
            nc.vector.tensor_copy(eself[:], e_cls[0:1, 0:1])
            nc.vector.tensor_scalar(corr[:], cv[0:1, 0:65], eself[:],
                                    float(NF - 1), op0=ALU.mult, op1=ALU.mult)
            nc.vector.tensor_tensor(out=clssum[:], in0=clssum[:], in1=corr[:],
                                    op=ALU.subtract)
            crinv = smallp.tile([1, 1], F32, tag="crinv")
            nc.vector.reciprocal(crinv[:], clssum[0:1, 64:65])
            clsa = smallp.tile([1, 64], F32, tag="clsa")
            nc.vector.tensor_scalar_mul(clsa[:], clssum[0:1, 0:64], crinv[:])
            ctp = ps_avt.tile([64, 197], F32, tag="avt")
            nc.tensor.transpose(ctp[0:64, 0:1], clsa[:], ident_f[0:1, 0:1])
            nc.vector.tensor_copy(aT[b][pb:pb + 64, 0:1], ctp[0:64, 0:1])

        # ---------------- Phase C: out projection --------------------------
        def c_sb(b, s0):
                nsb = min(512, N - s0)
                ysb = wrk.tile([128, 4, 512], F32, tag="ysb")
                for r0 in range(s0, s0 + nsb, 128):
                    nr = min(128, N - r0)
                    c = (r0 - s0) // 128
                    yp = ps_big.tile([128, 512], F32, tag="big")
                    nc.tensor.matmul(yp[0:nr, :], aT[b][:, r0:r0 + nr], wo_r[:],
                                     start=True, stop=True)
                    copy_alt(ysb[0:nr, c, :], yp[0:nr, :])
                nfull = nsb // 128
                if nfull:
                    nc.sync.dma_start(
                        y[b * N + s0: b * N + s0 + nfull * 128, :]
                        .rearrange("(c p) d -> p c d", p=128),
                        ysb[:, 0:nfull, :])
                if nsb % 128:
                    nc.sync.dma_start(
                        y[b * N + s0 + nfull * 128: b * N + s0 + nsb, :],
                        ysb[0:nsb % 128, nfull, :])

        def b_units(b):
            units = []
            states = [{}, {}]

            def setup(h, state):
                e_cls = smallp.tile([1, QCOLS], BF16, tag="ecls", name="e_cls")
                cv = smallp.tile([1, 65], BF16, tag="cv", name="cv")
                clsacc = smallp.tile([1, 65 * NF], F32, tag="clsacc",
                                     name="clsacc")
                pb = h * 64
                cvp = ps_at.tile([64, 197], BF16, tag="at", name="cvp")
                nc.tensor.transpose(cvp[0:1, 0:64], vT[b][pb:pb + 64, 0:1],
                                    ident_b[pb:pb + 64, pb:pb + 64])
                nc.vector.tensor_copy(cv[0:1, 0:64], cvp[0:1, 0:64])
                nc.vector.memset(cv[0:1, 64:65], 1.0)
                phase_s(b, h, e_cls)
                state.update(e_cls=e_cls, cv=cv, clsacc=clsacc,
                             veb=ve_prebuild(b, h))
            units.append(lambda: setup(0, states[0]))
            units.append(lambda: setup(1, states[1]))
            for f in range(NF):
                units.append(lambda f=f: b_stage1(b, 0, f, states[0]))
                units.append(lambda f=f: b_stage1(b, 1, f, states[1]))
                units.append(lambda f=f: b_stage2(b, 0, f, states[0]))
                units.append(lambda f=f: b_stage2(b, 1, f, states[1]))
            units.append(lambda: cls_final(b, 0, states[0]["e_cls"],
                                           states[0]["cv"], states[0]["clsacc"]))
            units.append(lambda: cls_final(b, 1, states[1]["e_cls"],
                                           states[1]["cv"], states[1]["clsacc"]))
            return units

        def interleave(main, aux, period):
            ai = 0
            for i, u in enumerate(main):
                u()
                if (i + 1) % period == 0 and ai < len(aux):
                    aux[ai]()
                    ai += 1
            while ai < len(aux):
                aux[ai]()
                ai += 1

        a_units_1 = [lambda r0=r0: a_sb(1, r0) for r0 in range(0, N, 512)]
        c_units_0 = [lambda s0=s0: c_sb(0, s0) for s0 in range(0, N, 512)]
        c_units_1 = [lambda s0=s0: c_sb(1, s0) for s0 in range(0, N, 512)]

        phase_a(0)
        interleave(b_units(0), a_units_1, 9)
        interleave(b_units(1), c_units_0, 9)
        for u in c_units_1:
            u()
    nc.finalize()
    return nc


_NC_CACHE = None


def _get_nc():
    global _NC_CACHE
    if _NC_CACHE is None:
        _NC_CACHE = build()
    return _NC_CACHE


def make_in_maps(x, W_qkv, W_out, b_out):
    x = np.asarray(x, dtype=np.float32)
    W_qkv = np.asarray(W_qkv, dtype=np.float32)
    W_out = np.asarray(W_out, dtype=np.float32)
    b_out = np.asarray(b_out, dtype=np.float32)
    in_maps = []
    for c in range(8):
        bg, hg = c // 4, c % 4
        cs = hg * 128
        m = {
            "x": np.ascontiguousarray(
                x[2 * bg:2 * bg + 2].reshape(NB * N, D)),
            "wq": np.ascontiguousarray(W_qkv[:, cs:cs + 128]),
            "wk": np.ascontiguousarray(W_qkv[:, 512 + cs:512 + cs + 128]),
            "wv": np.ascontiguousarray(W_qkv[:, 1024 + cs:1024 + cs + 128]),
            "wo": np.ascontiguousarray(W_out[cs:cs + 128, :]),
        }
        in_maps.append(m)
    return in_maps


def assemble(results, b_out):
    y = np.empty((B, N, D), dtype=np.float32)
    for bg in range(2):
        acc = results[bg * 4]["y"].copy()
        for hg in range(1, 4):
            acc += results[bg * 4 + hg]["y"]
        acc += np.asarray(b_out, dtype=np.float32)[None, :]
        y[2 * bg:2 * bg + 2] = acc.reshape(NB, N, D)
    return y


def kernel(x, W_qkv, W_out, b_out, f):
    assert int(f) == NF
    nc = _get_nc()
    in_maps = make_in_maps(x, W_qkv, W_out, b_out)
    res = bass_utils.run_bass_kernel_spmd(nc, in_maps, core_ids=list(range(8)))
    return assemble(res.results, b_out)


# revision 12
# speedup vs baseline: 1.2193x; 1.0731x over previous
"""TimeSformer divided-space attention (nn_Attention_3856880632239) on 8 TRN2 cores.

Sharding: core c = (batch-group bg = c//4) x (head-pair hg = c%4).
Each core handles 2 batches x 2 heads:
  - qkv projection in transposed layout (feature-on-partition), fp32r matmuls
  - per (b,h,frame): scoresT = k @ q^T (keys on partitions), exp (bf16),
    AV in natural orientation (queries on partitions) so softmax
    normalization is a per-partition scalar multiply, then PE-transpose back.
  - CLS query handled via query replication per frame section + partial-sum
    accumulation; CLS key handled via a batched rank-1 (K=1) matmul.
  - out projection with W_out row slice; partials summed on host.
"""
import numpy as np
from contextlib import ExitStack

import concourse.bass as bass
from concourse import bacc
import concourse.tile as tile
import concourse.mybir as mybir
from concourse import bass_utils
from concourse.masks import make_identity

F32 = mybir.dt.float32
F32R = mybir.dt.float32r
BF16 = mybir.dt.bfloat16
AF = mybir.ActivationFunctionType
ALU = mybir.AluOpType

B, N, D, H, DH, NF, NP = 4, 3137, 512, 8, 64, 16, 196
NB = 2            # batches per core
SEC = NP + 1      # 197: section = [cls | frame queries]
QCOLS = 3584      # sectioned query buffer (16*197=3152, padded to 7*512)
KCOLS = N         # global key/row space
SCALE = DH ** -0.5


def _sec_pieces(g0, g1):
    """Global row range [g0,g1) (g0>=1) -> (src_off, dst_col, len) pieces."""
    out = []
    g = g0
    while g < g1:
        s, j = (g - 1) // NP, (g - 1) % NP
        run = min(g1 - g, NP - j)
        out.append((g - g0, SEC * s + 1 + j, run))
        g += run
    return out


def build():
    nc = bacc.Bacc("TRN2", target_bir_lowering=False, debug=False, num_devices=8)
    x = nc.dram_tensor("x", [NB * N, D], F32, kind="ExternalInput")
    wq = nc.dram_tensor("wq", [D, 128], F32, kind="ExternalInput")
    wk = nc.dram_tensor("wk", [D, 128], F32, kind="ExternalInput")
    wv = nc.dram_tensor("wv", [D, 128], F32, kind="ExternalInput")
    wo = nc.dram_tensor("wo", [128, D], F32, kind="ExternalInput")
    y = nc.dram_tensor("y", [NB * N, D], F32, kind="ExternalOutput")

    with tile.TileContext(nc) as tc, ExitStack() as ctx:
        consts = ctx.enter_context(tc.tile_pool(name="consts", bufs=1))
        wrk = ctx.enter_context(tc.tile_pool(name="wrk", bufs=3))
        xtp = ctx.enter_context(tc.tile_pool(name="xtp", bufs=2))
        smallp = ctx.enter_context(tc.tile_pool(name="smallp", bufs=2))
        ps_big = ctx.enter_context(tc.tile_pool(name="ps_big", bufs=4, space="PSUM"))
        ps_avt = ctx.enter_context(tc.tile_pool(name="ps_avt", bufs=2, space="PSUM"))
        ps_at = ctx.enter_context(tc.tile_pool(name="ps_at", bufs=2, space="PSUM"))

        ident_f = consts.tile([128, 128], F32, tag="identf")
        make_identity(nc, ident_f[:])
        ident_b = consts.tile([128, 128], BF16, tag="identb")
        make_identity(nc, ident_b[:])

        # weights -> SBUF, rounded to f32r
        wq_f = consts.tile([128, 4, 128], F32, tag="wqf")
        wk_f = consts.tile([128, 4, 128], F32, tag="wkf")
        wv_f = consts.tile([128, 4, 128], F32, tag="wvf")
        for wt, wdr in ((wq_f, wq), (wk_f, wk), (wv_f, wv)):
            nc.sync.dma_start(wt[:], wdr[:].rearrange("(k p) c -> p k c", p=128))
        wq_r = consts.tile([128, 4, 128], F32R, tag="wqr")
        wk_r = consts.tile([128, 4, 128], F32R, tag="wkr")
        wv_r = consts.tile([128, 4, 128], F32R, tag="wvr")
        nc.scalar.copy(wq_r[:], wq_f[:])
        nc.scalar.copy(wk_r[:], wk_f[:])
        nc.scalar.copy(wv_r[:], wv_f[:])
        wo_f = consts.tile([128, D], F32, tag="wof")
        nc.sync.dma_start(wo_f[:], wo[:])
        wo_r = consts.tile([128, D], F32R, tag="wor")
        nc.scalar.copy(wo_r[:], wo_f[:])

        qT, kT, vT, aT = [], [], [], []
        for b in range(NB):
            qT.append(consts.tile([128, QCOLS], F32R, tag=f"qT{b}", name=f"qT{b}"))
            kT.append(consts.tile([128, QCOLS], F32R, tag=f"kT{b}", name=f"kT{b}"))
            vT.append(consts.tile([128, QCOLS], BF16, tag=f"vT{b}", name=f"vT{b}"))
            aT.append(consts.tile([128, KCOLS], F32R, tag=f"aT{b}", name=f"aT{b}"))
        zpad = consts.tile([128, QCOLS - 16 * SEC], F32, tag="zpad")
        nc.vector.memset(zpad[:], 0.0)
        for b in range(NB):
            nc.scalar.copy(qT[b][:, 16 * SEC:QCOLS], zpad[:])

        copy_flip = [0]

        def copy_alt(dst, src):
            copy_flip[0] ^= 1
            if copy_flip[0]:
                nc.scalar.copy(dst, src)
            else:
                nc.vector.tensor_copy(dst, src)

        # ---------------- Phase A: transpose x + qkv projection ------------
        def a_sb(b, r0):
                nsb = min(512, N - r0)
                xtt = xtp.tile([128, 4, 512], F32R, tag="xT")
                xa = wrk.tile([128, 4, 512], F32, tag="xa")
                nfull = nsb // 128
                if nfull:
                    nc.sync.dma_start(
                        xa[:, 0:nfull, :],
                        x[b * N + r0: b * N + r0 + nfull * 128, :]
                        .rearrange("(c p) d -> p c d", p=128))
                if nsb % 128:
                    nc.sync.dma_start(
                        xa[0:nsb % 128, nfull, :],
                        x[b * N + r0 + nfull * 128: b * N + r0 + nsb, :])
                for rr0 in range(r0, r0 + nsb, 128):
                    nr = min(128, N - rr0)
                    c = (rr0 - r0) // 128
                    pack = ps_big.tile([128, 512], F32, tag="big")
                    for k in range(4):
                        nc.tensor.transpose(
                            pack[:, k * 128: k * 128 + nr],
                            xa[0:nr, c, k * 128:(k + 1) * 128],
                            ident_f[0:nr, 0:nr])
                    off = rr0 - r0
                    copy_alt(
                        xtt[:, :, off:off + nr],
                        pack[:].rearrange("p (k c) -> p k c", k=4)[:, :, 0:nr])
                nmm = nsb + (nsb & 1)  # fp32r needs an even moving dim
                for wt, g in ((wq_r, 0), (wk_r, 1), (wv_r, 2)):
                    pp = ps_big.tile([128, 512], F32, tag="big")
                    for k in range(4):
                        nc.tensor.matmul(pp[:, 0:nmm], wt[:, k, :],
                                         xtt[:, k, 0:nmm],
                                         start=(k == 0), stop=(k == 3))
                    dst = (qT, kT, vT)[g]
                    cpy = nc.scalar.copy if g == 0 else nc.vector.tensor_copy
                    if r0 == 0:
                        for si in range(NF):
                            cpy(dst[b][:, SEC * si: SEC * si + 1], pp[:, 0:1])
                    g0 = max(r0, 1)
                    for src, dstc, ln in _sec_pieces(g0, r0 + nsb):
                        cpy(dst[b][:, dstc:dstc + ln],
                            pp[:, (g0 - r0) + src:(g0 - r0) + src + ln])

        def phase_a(b):
            for r0 in range(0, N, 512):
                a_sb(b, r0)

        # ---------------- Phase S: batched CLS-key scores ------------------
        # ---------------- Phase B: per-frame attention ---------------------
        def ve_prebuild(b, h):
            pb = h * 64
            veb = wrk.tile([128, NF * 130], BF16, tag="veb", name="veb", bufs=2)
            for f in range(NF):
                w0 = SEC * f
                vp = ps_avt.tile([128, 128], BF16, tag="avt", name="vp")
                nc.tensor.transpose(vp[0:128, 0:64], vT[b][pb:pb + 64, w0:w0 + 128],
                                    ident_b[pb:pb + 64, pb:pb + 64])
                nc.tensor.transpose(vp[0:69, 64:128],
                                    vT[b][pb:pb + 64, w0 + 128:w0 + 197],
                                    ident_b[pb:pb + 64, pb:pb + 64])
                o = 130 * f
                nc.vector.tensor_copy(
                    veb[0:128, o:o + 130].rearrange(
                        "p (g c) -> p g c", g=2)[:, :, 0:64],
                    vp[0:128, :].rearrange("p (g c) -> p g c", g=2))
            nc.gpsimd.memset(
                veb[0:128, :].rearrange("p (s c) -> p s c", s=NF)[:, :, 64:65], 1.0)
            nc.gpsimd.memset(
                veb[0:69, :].rearrange("p (s c) -> p s c", s=NF)[:, :, 129:130], 1.0)
            return veb

        def b_stage1(b, h, f, st):
            pb = h * 64
            w0 = SEC * f
            # scoresT: [keys, window-queries]; sectioned layout incl cls key
            sc = ps_big.tile([128, 512], F32, tag="big", name="sc")
            nc.tensor.matmul(sc[0:128, 0:256], kT[b][pb:pb + 64, w0:w0 + 128],
                             qT[b][pb:pb + 64, w0:w0 + 256], start=True, stop=True)
            nc.tensor.matmul(sc[0:69, 256:512], kT[b][pb:pb + 64, w0 + 128:w0 + 197],
                             qT[b][pb:pb + 64, w0:w0 + 256], start=True, stop=True)
            expT = wrk.tile([128, 394], BF16, tag="expT", name="expT")
            nc.scalar.activation(expT[0:128, 0:197], sc[0:128, 0:197],
                                 AF.Exp, scale=SCALE)
            nc.scalar.activation(expT[0:69, 197:394], sc[0:69, 256:453],
                                 AF.Exp, scale=SCALE)
            st["expT"] = expT

        def b_stage2(b, h, f, st):
            pb = h * 64
            w0 = SEC * f
            kc = 1 + NP * f
            clsacc = st["clsacc"]
            expT, veb = st["expT"], st["veb"]
            ve = veb[:, 130 * f:130 * f + 130]
            avp = ps_avt.tile([128, 130], F32, tag="avt", name="avp")
            nc.tensor.matmul(avp[0:128, 0:65], expT[0:128, 0:128], ve[0:128, 0:65],
                             start=True, stop=False)
            nc.tensor.matmul(avp[0:128, 0:65], expT[0:69, 197:325], ve[0:69, 65:130],
                             start=False, stop=True)
            nc.tensor.matmul(avp[0:69, 65:130], expT[0:128, 128:197], ve[0:128, 0:65],
                             start=True, stop=False)
            nc.tensor.matmul(avp[0:69, 65:130], expT[0:69, 325:394], ve[0:69, 65:130],
                             start=False, stop=True)
            # cls partial accumulation (row 0 = cls query)
            nc.vector.tensor_copy(clsacc[0:1, 65 * f:65 * f + 65], avp[0:1, 0:65])
            # normalize
            rinv = smallp.tile([128, 2], F32, tag="rinv")
            nc.vector.reciprocal(
                rinv[:], avp[0:128, :].rearrange("p (g c) -> p g c", g=2)[:, :, 64])
            ana = wrk.tile([128, 64], BF16, tag="ana")
            anb = wrk.tile([69, 64], BF16, tag="anb")
            nc.vector.tensor_scalar_mul(ana[:], avp[0:128, 0:64], rinv[:, 0:1])
            nc.vector.tensor_scalar_mul(anb[:], avp[0:69, 65:129], rinv[0:69, 1:2])
            atp = ps_at.tile([64, 197], BF16, tag="at")
            nc.tensor.transpose(atp[0:64, 0:128], ana[:], ident_b[:])
            nc.tensor.transpose(atp[0:64, 128:197], anb[:], ident_b[0:69, 0:69])
            nc.vector.tensor_copy(aT[b][pb:pb + 64, kc:kc + NP], atp[0:64, 1:197])

        def cls_final(b, h, expT, veb, clsacc):
            pb = h * 64
            clssum = smallp.tile([1, 65], F32, tag="clssum")
            nc.vector.reduce_sum(
                clssum[:], clsacc[0:1, :].rearrange("p (s c) -> p c s", s=NF),
                axis=mybir.AxisListType.X)
            corr = smallp.tile([1, 65], F32, tag="corr")
            eself = smallp.tile([1, 1], F32, tag="eself")
            nc.vector.tensor_copy(eself[:], expT[0:1, 0:1])
            nc.vector.tensor_scalar(corr[:], veb[0:1, 0:65], eself[:],
                                    float(NF - 1), op0=ALU.mult, op1=ALU.mult)
            nc.vector.tensor_tensor(out=clssum[:], in0=clssum[:], in1=corr[:],
                                    op=ALU.subtract)
            crinv = smallp.tile([1, 1], F32, tag="crinv")
            nc.vector.reciprocal(crinv[:], clssum[0:1, 64:65])
            clsa = smallp.tile([1, 64], F32, tag="clsa")
            nc.vector.tensor_scalar_mul(clsa[:], clssum[0:1, 0:64], crinv[:])
            ctp = ps_avt.tile([64, 197], F32, tag="avt")
            nc.tensor.transpose(ctp[0:64, 0:1], clsa[:], ident_f[0:1, 0:1])
            nc.vector.tensor_copy(aT[b][pb:pb + 64, 0:1], ctp[0:64, 0:1])

        # ---------------- Phase C: out projection --------------------------
        def c_sb(b, s0):
                nsb = min(512, N - s0)
                ysb = wrk.tile([128, 4, 512], F32, tag="ysb")
                for r0 in range(s0, s0 + nsb, 128):
                    nr = min(128, N - r0)
                    c = (r0 - s0) // 128
                    yp = ps_big.tile([128, 512], F32, tag="big")
                    nc.tensor.matmul(yp[0:nr, :], aT[b][:, r0:r0 + nr], wo_r[:],
                                     start=True, stop=True)
                    copy_alt(ysb[0:nr, c, :], yp[0:nr, :])
                nfull = nsb // 128
                if nfull:
                    nc.sync.dma_start(
                        y[b * N + s0: b * N + s0 + nfull * 128, :]
                        .rearrange("(c p) d -> p c d", p=128),
                        ysb[:, 0:nfull, :])
                if nsb % 128:
                    nc.sync.dma_start(
                        y[b * N + s0 + nfull * 128: b * N + s0 + nsb, :],
                        ysb[0:nsb % 128, nfull, :])

        def b_units(b):
            units = []
            states = [{}, {}]

            def setup(h, state):
                clsacc = smallp.tile([1, 65 * NF], F32, tag="clsacc",
                                     name="clsacc")
                state.update(clsacc=clsacc, veb=ve_prebuild(b, h))
            units.append(lambda: setup(0, states[0]))
            units.append(lambda: setup(1, states[1]))
            for f in range(NF):
                units.append(lambda f=f: b_stage1(b, 0, f, states[0]))
                units.append(lambda f=f: b_stage1(b, 1, f, states[1]))
                units.append(lambda f=f: b_stage2(b, 0, f, states[0]))
                units.append(lambda f=f: b_stage2(b, 1, f, states[1]))
            units.append(lambda: cls_final(b, 0, states[0]["expT"],
                                           states[0]["veb"], states[0]["clsacc"]))
            units.append(lambda: cls_final(b, 1, states[1]["expT"],
                                           states[1]["veb"], states[1]["clsacc"]))
            return units

        def interleave(main, aux, period):
            ai = 0
            for i, u in enumerate(main):
                u()
                if (i + 1) % period == 0 and ai < len(aux):
                    aux[ai]()
                    ai += 1
            while ai < len(aux):
                aux[ai]()
                ai += 1

        a_units_1 = [lambda r0=r0: a_sb(1, r0) for r0 in range(0, N, 512)]
        c_units_0 = [lambda s0=s0: c_sb(0, s0) for s0 in range(0, N, 512)]
        c_units_1 = [lambda s0=s0: c_sb(1, s0) for s0 in range(0, N, 512)]

        phase_a(0)
        interleave(b_units(0), a_units_1, 9)
        interleave(b_units(1), c_units_0, 9)
        for u in c_units_1:
            u()
    nc.finalize()
    return nc


_NC_CACHE = None


def _get_nc():
    global _NC_CACHE
    if _NC_CACHE is None:
        _NC_CACHE = build()
    return _NC_CACHE


def make_in_maps(x, W_qkv, W_out, b_out):
    x = np.asarray(x, dtype=np.float32)
    W_qkv = np.asarray(W_qkv, dtype=np.float32)
    W_out = np.asarray(W_out, dtype=np.float32)
    b_out = np.asarray(b_out, dtype=np.float32)
    in_maps = []
    for c in range(8):
        bg, hg = c // 4, c % 4
        cs = hg * 128
        m = {
            "x": np.ascontiguousarray(
                x[2 * bg:2 * bg + 2].reshape(NB * N, D)),
            "wq": np.ascontiguousarray(W_qkv[:, cs:cs + 128]),
            "wk": np.ascontiguousarray(W_qkv[:, 512 + cs:512 + cs + 128]),
            "wv": np.ascontiguousarray(W_qkv[:, 1024 + cs:1024 + cs + 128]),
            "wo": np.ascontiguousarray(W_out[cs:cs + 128, :]),
        }
        in_maps.append(m)
    return in_maps


def assemble(results, b_out):
    y = np.empty((B, N, D), dtype=np.float32)
    for bg in range(2):
        acc = results[bg * 4]["y"].copy()
        for hg in range(1, 4):
            acc += results[bg * 4 + hg]["y"]
        acc += np.asarray(b_out, dtype=np.float32)[None, :]
        y[2 * bg:2 * bg + 2] = acc.reshape(NB, N, D)
    return y


def kernel(x, W_qkv, W_out, b_out, f):
    assert int(f) == NF
    nc = _get_nc()
    in_maps = make_in_maps(x, W_qkv, W_out, b_out)
    res = bass_utils.run_bass_kernel_spmd(nc, in_maps, core_ids=list(range(8)))
    return assemble(res.results, b_out)


# revision 14
# speedup vs baseline: 1.2483x; 1.0238x over previous
"""TimeSformer divided-space attention (nn_Attention_3856880632239) on 8 TRN2 cores.

Sharding: core c = (batch-group bg = c//4) x (head-pair hg = c%4).
Each core handles 2 batches x 2 heads:
  - qkv projection in transposed layout (feature-on-partition), fp32r matmuls
  - per (b,h,frame): scoresT = k @ q^T (keys on partitions), exp (bf16),
    AV in natural orientation (queries on partitions) so softmax
    normalization is a per-partition scalar multiply, then PE-transpose back.
  - CLS query handled via query replication per frame section + partial-sum
    accumulation; CLS key handled via a batched rank-1 (K=1) matmul.
  - out projection with W_out row slice; partials summed on host.
"""
import numpy as np
from contextlib import ExitStack

import concourse.bass as bass
from concourse import bacc
import concourse.tile as tile
import concourse.mybir as mybir
from concourse import bass_utils
from concourse.masks import make_identity

F32 = mybir.dt.float32
F32R = mybir.dt.float32r
BF16 = mybir.dt.bfloat16
AF = mybir.ActivationFunctionType
ALU = mybir.AluOpType

B, N, D, H, DH, NF, NP = 4, 3137, 512, 8, 64, 16, 196
NB = 2            # batches per core
SEC = NP + 1      # 197: section = [cls | frame queries]
QCOLS = 3584      # sectioned query buffer (16*197=3152, padded to 7*512)
KCOLS = N         # global key/row space
SCALE = DH ** -0.5


def _sec_pieces(g0, g1):
    """Global row range [g0,g1) (g0>=1) -> (src_off, dst_col, len) pieces."""
    out = []
    g = g0
    while g < g1:
        s, j = (g - 1) // NP, (g - 1) % NP
        run = min(g1 - g, NP - j)
        out.append((g - g0, SEC * s + 1 + j, run))
        g += run
    return out


def build():
    nc = bacc.Bacc("TRN2", target_bir_lowering=False, debug=False, num_devices=8)
    x = nc.dram_tensor("x", [NB * N, D], F32, kind="ExternalInput")
    wq = nc.dram_tensor("wq", [D, 128], F32, kind="ExternalInput")
    wk = nc.dram_tensor("wk", [D, 128], F32, kind="ExternalInput")
    wv = nc.dram_tensor("wv", [D, 128], F32, kind="ExternalInput")
    wo = nc.dram_tensor("wo", [128, D], F32, kind="ExternalInput")
    y = nc.dram_tensor("y", [NB * N, D], F32, kind="ExternalOutput")

    with tile.TileContext(nc) as tc, ExitStack() as ctx:
        consts = ctx.enter_context(tc.tile_pool(name="consts", bufs=1))
        wrk = ctx.enter_context(tc.tile_pool(name="wrk", bufs=3))
        xtp = ctx.enter_context(tc.tile_pool(name="xtp", bufs=2))
        smallp = ctx.enter_context(tc.tile_pool(name="smallp", bufs=2))
        ps_big = ctx.enter_context(tc.tile_pool(name="ps_big", bufs=4, space="PSUM"))
        ps_avt = ctx.enter_context(tc.tile_pool(name="ps_avt", bufs=2, space="PSUM"))
        ps_at = ctx.enter_context(tc.tile_pool(name="ps_at", bufs=2, space="PSUM"))

        ident_f = consts.tile([128, 128], F32, tag="identf")
        make_identity(nc, ident_f[:])
        ident_b = consts.tile([128, 128], BF16, tag="identb")
        make_identity(nc, ident_b[:])

        # weights -> SBUF, rounded to f32r
        wq_f = consts.tile([128, 4, 128], F32, tag="wqf")
        wk_f = consts.tile([128, 4, 128], F32, tag="wkf")
        wv_f = consts.tile([128, 4, 128], F32, tag="wvf")
        for wt, wdr in ((wq_f, wq), (wk_f, wk), (wv_f, wv)):
            nc.sync.dma_start(wt[:], wdr[:].rearrange("(k p) c -> p k c", p=128))
        wq_r = consts.tile([128, 4, 128], F32R, tag="wqr")
        wk_r = consts.tile([128, 4, 128], F32R, tag="wkr")
        wv_r = consts.tile([128, 4, 128], F32R, tag="wvr")
        nc.scalar.copy(wq_r[:], wq_f[:])
        nc.scalar.copy(wk_r[:], wk_f[:])
        nc.scalar.copy(wv_r[:], wv_f[:])
        wo_f = consts.tile([128, D], F32, tag="wof")
        nc.sync.dma_start(wo_f[:], wo[:])
        wo_r = consts.tile([128, D], F32R, tag="wor")
        nc.scalar.copy(wo_r[:], wo_f[:])

        qT, kT, vT, aT = [], [], [], []
        for b in range(NB):
            qT.append(consts.tile([128, QCOLS], F32R, tag=f"qT{b}", name=f"qT{b}"))
            kT.append(consts.tile([128, QCOLS], F32R, tag=f"kT{b}", name=f"kT{b}"))
            vT.append(consts.tile([128, QCOLS], BF16, tag=f"vT{b}", name=f"vT{b}"))
            aT.append(consts.tile([128, KCOLS], F32R, tag=f"aT{b}", name=f"aT{b}"))
        zpad = consts.tile([128, QCOLS - 16 * SEC], F32, tag="zpad")
        nc.vector.memset(zpad[:], 0.0)
        for b in range(NB):
            nc.scalar.copy(qT[b][:, 16 * SEC:QCOLS], zpad[:])

        copy_flip = [0]

        def copy_alt(dst, src):
            copy_flip[0] ^= 1
            if copy_flip[0]:
                nc.scalar.copy(dst, src)
            else:
                nc.vector.tensor_copy(dst, src)

        # ---------------- Phase A: transpose x + qkv projection ------------
        def a_sb(b, r0):
                nsb = min(512, N - r0)
                xtt = xtp.tile([128, 4, 512], F32R, tag="xT")
                xa = wrk.tile([128, 4, 512], F32, tag="xa")
                nfull = nsb // 128
                if nfull:
                    nc.sync.dma_start(
                        xa[:, 0:nfull, :],
                        x[b * N + r0: b * N + r0 + nfull * 128, :]
                        .rearrange("(c p) d -> p c d", p=128))
                if nsb % 128:
                    nc.sync.dma_start(
                        xa[0:nsb % 128, nfull, :],
                        x[b * N + r0 + nfull * 128: b * N + r0 + nsb, :])
                for rr0 in range(r0, r0 + nsb, 128):
                    nr = min(128, N - rr0)
                    c = (rr0 - r0) // 128
                    pack = ps_big.tile([128, 512], F32, tag="big")
                    for k in range(4):
                        nc.tensor.transpose(
                            pack[:, k * 128: k * 128 + nr],
                            xa[0:nr, c, k * 128:(k + 1) * 128],
                            ident_f[0:nr, 0:nr])
                    off = rr0 - r0
                    copy_alt(
                        xtt[:, :, off:off + nr],
                        pack[:].rearrange("p (k c) -> p k c", k=4)[:, :, 0:nr])
                nmm = nsb + (nsb & 1)  # fp32r needs an even moving dim
                for wt, g in ((wq_r, 0), (wk_r, 1), (wv_r, 2)):
                    pp = ps_big.tile([128, 512], F32, tag="big")
                    for k in range(4):
                        nc.tensor.matmul(pp[:, 0:nmm], wt[:, k, :],
                                         xtt[:, k, 0:nmm],
                                         start=(k == 0), stop=(k == 3))
                    dst = (qT, kT, vT)[g]
                    cpy = nc.scalar.copy if g == 0 else nc.vector.tensor_copy
                    if r0 == 0:
                        for si in range(NF):
                            cpy(dst[b][:, SEC * si: SEC * si + 1], pp[:, 0:1])
                    g0 = max(r0, 1)
                    for src, dstc, ln in _sec_pieces(g0, r0 + nsb):
                        cpy(dst[b][:, dstc:dstc + ln],
                            pp[:, (g0 - r0) + src:(g0 - r0) + src + ln])

        def phase_a(b):
            for r0 in range(0, N, 512):
                a_sb(b, r0)

        # ---------------- Phase S: batched CLS-key scores ------------------
        # ---------------- Phase B: per-frame attention ---------------------
        def ve_prebuild(b, h):
            pb = h * 64
            veb = wrk.tile([128, NF * 130], BF16, tag="veb", name="veb", bufs=2)
            for f in range(NF):
                w0 = SEC * f
                vp = ps_avt.tile([128, 128], BF16, tag="avt", name="vp")
                nc.tensor.transpose(vp[0:128, 0:64], vT[b][pb:pb + 64, w0:w0 + 128],
                                    ident_b[pb:pb + 64, pb:pb + 64])
                nc.tensor.transpose(vp[0:69, 64:128],
                                    vT[b][pb:pb + 64, w0 + 128:w0 + 197],
                                    ident_b[pb:pb + 64, pb:pb + 64])
                o = 130 * f
                nc.vector.tensor_copy(
                    veb[0:128, o:o + 130].rearrange(
                        "p (g c) -> p g c", g=2)[:, :, 0:64],
                    vp[0:128, :].rearrange("p (g c) -> p g c", g=2))
            nc.gpsimd.memset(
                veb[0:128, :].rearrange("p (s c) -> p s c", s=NF)[:, :, 64:65], 1.0)
            nc.gpsimd.memset(
                veb[0:69, :].rearrange("p (s c) -> p s c", s=NF)[:, :, 129:130], 1.0)
            return veb

        def b_stage1(b, h, f, st):
            pb = h * 64
            w0 = SEC * f
            # scoresT: [keys, window-queries]; sectioned layout incl cls key
            sc = ps_big.tile([128, 512], F32, tag="big", name="sc")
            nc.tensor.matmul(sc[0:128, 0:256], kT[b][pb:pb + 64, w0:w0 + 128],
                             qT[b][pb:pb + 64, w0:w0 + 256], start=True, stop=True)
            nc.tensor.matmul(sc[0:69, 256:512], kT[b][pb:pb + 64, w0 + 128:w0 + 197],
                             qT[b][pb:pb + 64, w0:w0 + 256], start=True, stop=True)
            expT = wrk.tile([128, 394], BF16, tag="expT", name="expT", bufs=5)
            nc.scalar.activation(expT[0:128, 0:197], sc[0:128, 0:197],
                                 AF.Exp, scale=SCALE)
            nc.scalar.activation(expT[0:69, 197:394], sc[0:69, 256:453],
                                 AF.Exp, scale=SCALE)
            st["expT"] = expT

        def b_stage2(b, h, f, st):
            pb = h * 64
            w0 = SEC * f
            kc = 1 + NP * f
            clsacc = st["clsacc"]
            expT, veb = st["expT"], st["veb"]
            ve = veb[:, 130 * f:130 * f + 130]
            avp = ps_avt.tile([128, 130], F32, tag="avt", name="avp")
            nc.tensor.matmul(avp[0:128, 0:65], expT[0:128, 0:128], ve[0:128, 0:65],
                             start=True, stop=False)
            nc.tensor.matmul(avp[0:128, 0:65], expT[0:69, 197:325], ve[0:69, 65:130],
                             start=False, stop=True)
            nc.tensor.matmul(avp[0:69, 65:130], expT[0:128, 128:197], ve[0:128, 0:65],
                             start=True, stop=False)
            nc.tensor.matmul(avp[0:69, 65:130], expT[0:69, 325:394], ve[0:69, 65:130],
                             start=False, stop=True)
            # cls partial accumulation (row 0 = cls query)
            nc.vector.tensor_copy(clsacc[0:1, 65 * f:65 * f + 65], avp[0:1, 0:65])
            # normalize
            rinv = smallp.tile([128, 2], F32, tag="rinv")
            nc.vector.reciprocal(
                rinv[:], avp[0:128, :].rearrange("p (g c) -> p g c", g=2)[:, :, 64])
            ana = wrk.tile([128, 64], BF16, tag="ana")
            anb = wrk.tile([69, 64], BF16, tag="anb")
            nc.vector.tensor_scalar_mul(ana[:], avp[0:128, 0:64], rinv[:, 0:1])
            nc.vector.tensor_scalar_mul(anb[:], avp[0:69, 65:129], rinv[0:69, 1:2])
            atp = ps_at.tile([64, 197], BF16, tag="at")
            nc.tensor.transpose(atp[0:64, 0:128], ana[:], ident_b[:])
            nc.tensor.transpose(atp[0:64, 128:197], anb[:], ident_b[0:69, 0:69])
            nc.vector.tensor_copy(aT[b][pb:pb + 64, kc:kc + NP], atp[0:64, 1:197])

        def cls_final(b, h, expT, veb, clsacc):
            pb = h * 64
            clssum = smallp.tile([1, 65], F32, tag="clssum")
            nc.vector.reduce_sum(
                clssum[:], clsacc[0:1, :].rearrange("p (s c) -> p c s", s=NF),
                axis=mybir.AxisListType.X)
            corr = smallp.tile([1, 65], F32, tag="corr")
            eself = smallp.tile([1, 1], F32, tag="eself")
            nc.vector.tensor_copy(eself[:], expT[0:1, 0:1])
            nc.vector.tensor_scalar(corr[:], veb[0:1, 0:65], eself[:],
                                    float(NF - 1), op0=ALU.mult, op1=ALU.mult)
            nc.vector.tensor_tensor(out=clssum[:], in0=clssum[:], in1=corr[:],
                                    op=ALU.subtract)
            crinv = smallp.tile([1, 1], F32, tag="crinv")
            nc.vector.reciprocal(crinv[:], clssum[0:1, 64:65])
            clsa = smallp.tile([1, 64], F32, tag="clsa")
            nc.vector.tensor_scalar_mul(clsa[:], clssum[0:1, 0:64], crinv[:])
            ctp = ps_avt.tile([64, 197], F32, tag="avt")
            nc.tensor.transpose(ctp[0:64, 0:1], clsa[:], ident_f[0:1, 0:1])
            nc.vector.tensor_copy(aT[b][pb:pb + 64, 0:1], ctp[0:64, 0:1])

        # ---------------- Phase C: out projection --------------------------
        def c_sb(b, s0):
                nsb = min(512, N - s0)
                ysb = wrk.tile([128, 4, 512], F32, tag="ysb")
                for r0 in range(s0, s0 + nsb, 128):
                    nr = min(128, N - r0)
                    c = (r0 - s0) // 128
                    yp = ps_big.tile([128, 512], F32, tag="big")
                    nc.tensor.matmul(yp[0:nr, :], aT[b][:, r0:r0 + nr], wo_r[:],
                                     start=True, stop=True)
                    copy_alt(ysb[0:nr, c, :], yp[0:nr, :])
                nfull = nsb // 128
                if nfull:
                    nc.sync.dma_start(
                        y[b * N + s0: b * N + s0 + nfull * 128, :]
                        .rearrange("(c p) d -> p c d", p=128),
                        ysb[:, 0:nfull, :])
                if nsb % 128:
                    nc.sync.dma_start(
                        y[b * N + s0 + nfull * 128: b * N + s0 + nsb, :],
                        ysb[0:nsb % 128, nfull, :])

        def b_units(b):
            units = []
            states = [{}, {}]

            def setup(h, state):
                clsacc = smallp.tile([1, 65 * NF], F32, tag="clsacc",
                                     name="clsacc")
                state.update(clsacc=clsacc, veb=ve_prebuild(b, h))
            units.append(lambda: setup(0, states[0]))
            units.append(lambda: setup(1, states[1]))
            # 2-deep software pipeline: stage2 lags stage1 by one frame
            exp_hist = [{}, {}]

            def s1(h, f):
                b_stage1(b, h, f, states[h])
                exp_hist[h][f] = states[h]["expT"]

            def s2(h, f):
                states[h]["expT"] = exp_hist[h].pop(f)
                b_stage2(b, h, f, states[h])

            for f in range(NF):
                units.append(lambda f=f: s1(0, f))
                units.append(lambda f=f: s1(1, f))
                if f > 0:
                    units.append(lambda f=f: s2(0, f - 1))
                    units.append(lambda f=f: s2(1, f - 1))
            units.append(lambda: s2(0, NF - 1))
            units.append(lambda: s2(1, NF - 1))
            units.append(lambda: cls_final(b, 0, states[0]["expT"],
                                           states[0]["veb"], states[0]["clsacc"]))
            units.append(lambda: cls_final(b, 1, states[1]["expT"],
                                           states[1]["veb"], states[1]["clsacc"]))
            return units

        def interleave(main, aux, period):
            ai = 0
            for i, u in enumerate(main):
                u()
                if (i + 1) % period == 0 and ai < len(aux):
                    aux[ai]()
                    ai += 1
            while ai < len(aux):
                aux[ai]()
                ai += 1

        a_units_1 = [lambda r0=r0: a_sb(1, r0) for r0 in range(0, N, 512)]
        c_units_0 = [lambda s0=s0: c_sb(0, s0) for s0 in range(0, N, 512)]
        c_units_1 = [lambda s0=s0: c_sb(1, s0) for s0 in range(0, N, 512)]

        phase_a(0)
        interleave(b_units(0), a_units_1, 9)
        interleave(b_units(1), c_units_0, 9)
        for u in c_units_1:
            u()
    nc.finalize()
    return nc


_NC_CACHE = None


def _get_nc():
    global _NC_CACHE
    if _NC_CACHE is None:
        _NC_CACHE = build()
    return _NC_CACHE


def make_in_maps(x, W_qkv, W_out, b_out):
    x = np.asarray(x, dtype=np.float32)
    W_qkv = np.asarray(W_qkv, dtype=np.float32)
    W_out = np.asarray(W_out, dtype=np.float32)
    b_out = np.asarray(b_out, dtype=np.float32)
    in_maps = []
    for c in range(8):
        bg, hg = c // 4, c % 4
        cs = hg * 128
        m = {
            "x": np.ascontiguousarray(
                x[2 * bg:2 * bg + 2].reshape(NB * N, D)),
            "wq": np.ascontiguousarray(W_qkv[:, cs:cs + 128]),
            "wk": np.ascontiguousarray(W_qkv[:, 512 + cs:512 + cs + 128]),
            "wv": np.ascontiguousarray(W_qkv[:, 1024 + cs:1024 + cs + 128]),
            "wo": np.ascontiguousarray(W_out[cs:cs + 128, :]),
        }
        in_maps.append(m)
    return in_maps


def assemble(results, b_out):
    y = np.empty((B, N, D), dtype=np.float32)
    for bg in range(2):
        acc = results[bg * 4]["y"].copy()
        for hg in range(1, 4):
            acc += results[bg * 4 + hg]["y"]
        acc += np.asarray(b_out, dtype=np.float32)[None, :]
        y[2 * bg:2 * bg + 2] = acc.reshape(NB, N, D)
    return y


def kernel(x, W_qkv, W_out, b_out, f):
    assert int(f) == NF
    nc = _get_nc()
    in_maps = make_in_maps(x, W_qkv, W_out, b_out)
    res = bass_utils.run_bass_kernel_spmd(nc, in_maps, core_ids=list(range(8)))
    return assemble(res.results, b_out)


# revision 15
# speedup vs baseline: 1.3549x; 1.0853x over previous
"""TimeSformer divided-space attention (nn_Attention_3856880632239) on 8 TRN2 cores.

Sharding: core c = (batch-group bg = c//4) x (head-pair hg = c%4).
Each core handles 2 batches x 2 heads:
  - qkv projection in transposed layout (feature-on-partition), fp32r matmuls
  - per (b,h,frame): scoresT = k @ q^T (keys on partitions), exp (bf16),
    AV in natural orientation (queries on partitions) so softmax
    normalization is a per-partition scalar multiply, then PE-transpose back.
  - CLS query handled via query replication per frame section + partial-sum
    accumulation; CLS key handled via a batched rank-1 (K=1) matmul.
  - out projection with W_out row slice; partials summed on host.
"""
import numpy as np
import ml_dtypes
from contextlib import ExitStack

import concourse.bass as bass
from concourse import bacc
import concourse.tile as tile
import concourse.mybir as mybir
from concourse import bass_utils
from concourse.masks import make_identity

F32 = mybir.dt.float32
F32R = mybir.dt.float32r
BF16 = mybir.dt.bfloat16
AF = mybir.ActivationFunctionType
ALU = mybir.AluOpType

B, N, D, H, DH, NF, NP = 4, 3137, 512, 8, 64, 16, 196
NB = 2            # batches per core
SEC = NP + 1      # 197: section = [cls | frame queries]
QCOLS = 3584      # sectioned query buffer (16*197=3152, padded to 7*512)
KCOLS = N         # global key/row space
SCALE = DH ** -0.5


def _sec_pieces(g0, g1):
    """Global row range [g0,g1) (g0>=1) -> (src_off, dst_col, len) pieces."""
    out = []
    g = g0
    while g < g1:
        s, j = (g - 1) // NP, (g - 1) % NP
        run = min(g1 - g, NP - j)
        out.append((g - g0, SEC * s + 1 + j, run))
        g += run
    return out


def build():
    nc = bacc.Bacc("TRN2", target_bir_lowering=False, debug=False, num_devices=8)
    x = nc.dram_tensor("x", [NB * N, D], BF16, kind="ExternalInput")
    wq = nc.dram_tensor("wq", [D, 128], F32, kind="ExternalInput")
    wk = nc.dram_tensor("wk", [D, 128], F32, kind="ExternalInput")
    wv = nc.dram_tensor("wv", [D, 128], F32, kind="ExternalInput")
    wo = nc.dram_tensor("wo", [128, D], F32, kind="ExternalInput")
    y = nc.dram_tensor("y", [NB * N, D], F32, kind="ExternalOutput")

    with tile.TileContext(nc) as tc, ExitStack() as ctx:
        consts = ctx.enter_context(tc.tile_pool(name="consts", bufs=1))
        wrk = ctx.enter_context(tc.tile_pool(name="wrk", bufs=3))
        xtp = ctx.enter_context(tc.tile_pool(name="xtp", bufs=2))
        smallp = ctx.enter_context(tc.tile_pool(name="smallp", bufs=2))
        ps_big = ctx.enter_context(tc.tile_pool(name="ps_big", bufs=4, space="PSUM"))
        ps_avt = ctx.enter_context(tc.tile_pool(name="ps_avt", bufs=2, space="PSUM"))
        ps_at = ctx.enter_context(tc.tile_pool(name="ps_at", bufs=2, space="PSUM"))

        ident_f = consts.tile([128, 128], F32, tag="identf")
        make_identity(nc, ident_f[:])
        ident_b = consts.tile([128, 128], BF16, tag="identb")
        make_identity(nc, ident_b[:])

        # weights -> SBUF, rounded to f32r
        wq_f = consts.tile([128, 4, 128], F32, tag="wqf")
        wk_f = consts.tile([128, 4, 128], F32, tag="wkf")
        wv_f = consts.tile([128, 4, 128], F32, tag="wvf")
        for wt, wdr in ((wq_f, wq), (wk_f, wk), (wv_f, wv)):
            nc.sync.dma_start(wt[:], wdr[:].rearrange("(k p) c -> p k c", p=128))
        wq_r = consts.tile([128, 4, 128], BF16, tag="wqr")
        wk_r = consts.tile([128, 4, 128], BF16, tag="wkr")
        wv_r = consts.tile([128, 4, 128], BF16, tag="wvr")
        nc.scalar.copy(wq_r[:], wq_f[:])
        nc.scalar.copy(wk_r[:], wk_f[:])
        nc.scalar.copy(wv_r[:], wv_f[:])
        wo_f = consts.tile([128, D], F32, tag="wof")
        nc.sync.dma_start(wo_f[:], wo[:])
        wo_r = consts.tile([128, D], F32R, tag="wor")
        nc.scalar.copy(wo_r[:], wo_f[:])

        qT, kT, vT, aT = [], [], [], []
        for b in range(NB):
            qT.append(consts.tile([128, QCOLS], F32R, tag=f"qT{b}", name=f"qT{b}"))
            kT.append(consts.tile([128, QCOLS], F32R, tag=f"kT{b}", name=f"kT{b}"))
            vT.append(consts.tile([128, QCOLS], BF16, tag=f"vT{b}", name=f"vT{b}"))
            aT.append(consts.tile([128, KCOLS], F32R, tag=f"aT{b}", name=f"aT{b}"))
        zpad = consts.tile([128, QCOLS - 16 * SEC], F32, tag="zpad")
        nc.vector.memset(zpad[:], 0.0)
        for b in range(NB):
            nc.scalar.copy(qT[b][:, 16 * SEC:QCOLS], zpad[:])

        copy_flip = [0]

        def copy_alt(dst, src):
            copy_flip[0] ^= 1
            if copy_flip[0]:
                nc.scalar.copy(dst, src)
            else:
                nc.vector.tensor_copy(dst, src)

        # ---------------- Phase A: transpose x + qkv projection ------------
        def a_sb(b, r0):
                nsb = min(512, N - r0)
                xtt = xtp.tile([128, 4, 512], BF16, tag="xT")
                xa = wrk.tile([128, 4, 512], BF16, tag="xa")
                nfull = nsb // 128
                if nfull:
                    nc.sync.dma_start(
                        xa[:, 0:nfull, :],
                        x[b * N + r0: b * N + r0 + nfull * 128, :]
                        .rearrange("(c p) d -> p c d", p=128))
                if nsb % 128:
                    nc.sync.dma_start(
                        xa[0:nsb % 128, nfull, :],
                        x[b * N + r0 + nfull * 128: b * N + r0 + nsb, :])
                for rr0 in range(r0, r0 + nsb, 128):
                    nr = min(128, N - rr0)
                    c = (rr0 - r0) // 128
                    pack = ps_big.tile([128, 512], BF16, tag="big")
                    for k in range(4):
                        nc.tensor.transpose(
                            pack[:, k * 128: k * 128 + nr],
                            xa[0:nr, c, k * 128:(k + 1) * 128],
                            ident_b[0:nr, 0:nr])
                    off = rr0 - r0
                    copy_alt(
                        xtt[:, :, off:off + nr],
                        pack[:].rearrange("p (k c) -> p k c", k=4)[:, :, 0:nr])
                nmm = nsb + (nsb & 1)  # fp32r needs an even moving dim
                for wt, g in ((wq_r, 0), (wk_r, 1), (wv_r, 2)):
                    pp = ps_big.tile([128, 512], F32, tag="big")
                    for k in range(4):
                        nc.tensor.matmul(pp[:, 0:nmm], wt[:, k, :],
                                         xtt[:, k, 0:nmm],
                                         start=(k == 0), stop=(k == 3))
                    dst = (qT, kT, vT)[g]
                    cpy = nc.scalar.copy if g == 0 else nc.vector.tensor_copy
                    if r0 == 0:
                        for si in range(NF):
                            cpy(dst[b][:, SEC * si: SEC * si + 1], pp[:, 0:1])
                    g0 = max(r0, 1)
                    for src, dstc, ln in _sec_pieces(g0, r0 + nsb):
                        cpy(dst[b][:, dstc:dstc + ln],
                            pp[:, (g0 - r0) + src:(g0 - r0) + src + ln])

        def phase_a(b):
            for r0 in range(0, N, 512):
                a_sb(b, r0)

        # ---------------- Phase S: batched CLS-key scores ------------------
        # ---------------- Phase B: per-frame attention ---------------------
        def ve_prebuild(b, h):
            pb = h * 64
            veb = wrk.tile([128, NF * 130], BF16, tag="veb", name="veb", bufs=2)
            for f in range(NF):
                w0 = SEC * f
                vp = ps_avt.tile([128, 128], BF16, tag="avt", name="vp")
                nc.tensor.transpose(vp[0:128, 0:64], vT[b][pb:pb + 64, w0:w0 + 128],
                                    ident_b[pb:pb + 64, pb:pb + 64])
                nc.tensor.transpose(vp[0:69, 64:128],
                                    vT[b][pb:pb + 64, w0 + 128:w0 + 197],
                                    ident_b[pb:pb + 64, pb:pb + 64])
                o = 130 * f
                nc.vector.tensor_copy(
                    veb[0:128, o:o + 130].rearrange(
                        "p (g c) -> p g c", g=2)[:, :, 0:64],
                    vp[0:128, :].rearrange("p (g c) -> p g c", g=2))
            nc.gpsimd.memset(
                veb[0:128, :].rearrange("p (s c) -> p s c", s=NF)[:, :, 64:65], 1.0)
            nc.gpsimd.memset(
                veb[0:69, :].rearrange("p (s c) -> p s c", s=NF)[:, :, 129:130], 1.0)
            return veb

        def b_stage1(b, h, f, st):
            pb = h * 64
            w0 = SEC * f
            # scoresT: [keys, window-queries]; sectioned layout incl cls key
            sc = ps_big.tile([128, 512], F32, tag="big", name="sc")
            nc.tensor.matmul(sc[0:128, 0:256], kT[b][pb:pb + 64, w0:w0 + 128],
                             qT[b][pb:pb + 64, w0:w0 + 256], start=True, stop=True)
            nc.tensor.matmul(sc[0:69, 256:512], kT[b][pb:pb + 64, w0 + 128:w0 + 197],
                             qT[b][pb:pb + 64, w0:w0 + 256], start=True, stop=True)
            expT = wrk.tile([128, 394], BF16, tag="expT", name="expT", bufs=5)
            nc.scalar.activation(expT[0:128, 0:197], sc[0:128, 0:197],
                                 AF.Exp, scale=SCALE)
            nc.scalar.activation(expT[0:69, 197:394], sc[0:69, 256:453],
                                 AF.Exp, scale=SCALE)
            st["expT"] = expT

        def b_stage2(b, h, f, st):
            pb = h * 64
            w0 = SEC * f
            kc = 1 + NP * f
            clsacc = st["clsacc"]
            expT, veb = st["expT"], st["veb"]
            ve = veb[:, 130 * f:130 * f + 130]
            avp = ps_avt.tile([128, 130], F32, tag="avt", name="avp")
            nc.tensor.matmul(avp[0:128, 0:65], expT[0:128, 0:128], ve[0:128, 0:65],
                             start=True, stop=False)
            nc.tensor.matmul(avp[0:128, 0:65], expT[0:69, 197:325], ve[0:69, 65:130],
                             start=False, stop=True)
            nc.tensor.matmul(avp[0:69, 65:130], expT[0:128, 128:197], ve[0:128, 0:65],
                             start=True, stop=False)
            nc.tensor.matmul(avp[0:69, 65:130], expT[0:69, 325:394], ve[0:69, 65:130],
                             start=False, stop=True)
            # cls partial accumulation (row 0 = cls query)
            nc.vector.tensor_copy(clsacc[0:1, 65 * f:65 * f + 65], avp[0:1, 0:65])
            # normalize
            rinv = smallp.tile([128, 2], F32, tag="rinv")
            nc.vector.reciprocal(
                rinv[:], avp[0:128, :].rearrange("p (g c) -> p g c", g=2)[:, :, 64])
            ana = wrk.tile([128, 64], BF16, tag="ana")
            anb = wrk.tile([69, 64], BF16, tag="anb")
            nc.vector.tensor_scalar_mul(ana[:], avp[0:128, 0:64], rinv[:, 0:1])
            nc.vector.tensor_scalar_mul(anb[:], avp[0:69, 65:129], rinv[0:69, 1:2])
            atp = ps_at.tile([64, 197], BF16, tag="at")
            nc.tensor.transpose(atp[0:64, 0:128], ana[:], ident_b[:])
            nc.tensor.transpose(atp[0:64, 128:197], anb[:], ident_b[0:69, 0:69])
            nc.vector.tensor_copy(aT[b][pb:pb + 64, kc:kc + NP], atp[0:64, 1:197])

        def cls_final(b, h, expT, veb, clsacc):
            pb = h * 64
            clssum = smallp.tile([1, 65], F32, tag="clssum")
            nc.vector.reduce_sum(
                clssum[:], clsacc[0:1, :].rearrange("p (s c) -> p c s", s=NF),
                axis=mybir.AxisListType.X)
            corr = smallp.tile([1, 65], F32, tag="corr")
            eself = smallp.tile([1, 1], F32, tag="eself")
            nc.vector.tensor_copy(eself[:], expT[0:1, 0:1])
            nc.vector.tensor_scalar(corr[:], veb[0:1, 0:65], eself[:],
                                    float(NF - 1), op0=ALU.mult, op1=ALU.mult)
            nc.vector.tensor_tensor(out=clssum[:], in0=clssum[:], in1=corr[:],
                                    op=ALU.subtract)
            crinv = smallp.tile([1, 1], F32, tag="crinv")
            nc.vector.reciprocal(crinv[:], clssum[0:1, 64:65])
            clsa = smallp.tile([1, 64], F32, tag="clsa")
            nc.vector.tensor_scalar_mul(clsa[:], clssum[0:1, 0:64], crinv[:])
            ctp = ps_avt.tile([64, 197], F32, tag="avt")
            nc.tensor.transpose(ctp[0:64, 0:1], clsa[:], ident_f[0:1, 0:1])
            nc.vector.tensor_copy(aT[b][pb:pb + 64, 0:1], ctp[0:64, 0:1])

        # ---------------- Phase C: out projection --------------------------
        def c_sb(b, s0):
                nsb = min(512, N - s0)
                ysb = wrk.tile([128, 4, 512], F32, tag="ysb")
                for r0 in range(s0, s0 + nsb, 128):
                    nr = min(128, N - r0)
                    c = (r0 - s0) // 128
                    yp = ps_big.tile([128, 512], F32, tag="big")
                    nc.tensor.matmul(yp[0:nr, :], aT[b][:, r0:r0 + nr], wo_r[:],
                                     start=True, stop=True)
                    copy_alt(ysb[0:nr, c, :], yp[0:nr, :])
                nfull = nsb // 128
                if nfull:
                    nc.sync.dma_start(
                        y[b * N + s0: b * N + s0 + nfull * 128, :]
                        .rearrange("(c p) d -> p c d", p=128),
                        ysb[:, 0:nfull, :])
                if nsb % 128:
                    nc.sync.dma_start(
                        y[b * N + s0 + nfull * 128: b * N + s0 + nsb, :],
                        ysb[0:nsb % 128, nfull, :])

        def b_units(b):
            units = []
            states = [{}, {}]

            def setup(h, state):
                clsacc = smallp.tile([1, 65 * NF], F32, tag="clsacc",
                                     name="clsacc")
                state.update(clsacc=clsacc, veb=ve_prebuild(b, h))
            units.append(lambda: setup(0, states[0]))
            units.append(lambda: setup(1, states[1]))
            # 2-deep software pipeline: stage2 lags stage1 by one frame
            exp_hist = [{}, {}]

            def s1(h, f):
                b_stage1(b, h, f, states[h])
                exp_hist[h][f] = states[h]["expT"]

            def s2(h, f):
                states[h]["expT"] = exp_hist[h].pop(f)
                b_stage2(b, h, f, states[h])

            for f in range(NF):
                units.append(lambda f=f: s1(0, f))
                units.append(lambda f=f: s1(1, f))
                if f > 0:
                    units.append(lambda f=f: s2(0, f - 1))
                    units.append(lambda f=f: s2(1, f - 1))
            units.append(lambda: s2(0, NF - 1))
            units.append(lambda: s2(1, NF - 1))
            units.append(lambda: cls_final(b, 0, states[0]["expT"],
                                           states[0]["veb"], states[0]["clsacc"]))
            units.append(lambda: cls_final(b, 1, states[1]["expT"],
                                           states[1]["veb"], states[1]["clsacc"]))
            return units

        def interleave(main, aux, period):
            ai = 0
            for i, u in enumerate(main):
                u()
                if (i + 1) % period == 0 and ai < len(aux):
                    aux[ai]()
                    ai += 1
            while ai < len(aux):
                aux[ai]()
                ai += 1

        a_units_1 = [lambda r0=r0: a_sb(1, r0) for r0 in range(0, N, 512)]
        c_units_0 = [lambda s0=s0: c_sb(0, s0) for s0 in range(0, N, 512)]
        c_units_1 = [lambda s0=s0: c_sb(1, s0) for s0 in range(0, N, 512)]

        phase_a(0)
        interleave(b_units(0), a_units_1, 9)
        interleave(b_units(1), c_units_0, 9)
        for u in c_units_1:
            u()
    nc.finalize()
    return nc


_NC_CACHE = None


def _get_nc():
    global _NC_CACHE
    if _NC_CACHE is None:
        _NC_CACHE = build()
    return _NC_CACHE


def make_in_maps(x, W_qkv, W_out, b_out):
    x = np.asarray(x, dtype=np.float32)
    W_qkv = np.asarray(W_qkv, dtype=np.float32)
    W_out = np.asarray(W_out, dtype=np.float32)
    b_out = np.asarray(b_out, dtype=np.float32)
    in_maps = []
    for c in range(8):
        bg, hg = c // 4, c % 4
        cs = hg * 128
        m = {
            "x": np.ascontiguousarray(
                x[2 * bg:2 * bg + 2].reshape(NB * N, D)).astype(
                    ml_dtypes.bfloat16),
            "wq": np.ascontiguousarray(W_qkv[:, cs:cs + 128]),
            "wk": np.ascontiguousarray(W_qkv[:, 512 + cs:512 + cs + 128]),
            "wv": np.ascontiguousarray(W_qkv[:, 1024 + cs:1024 + cs + 128]),
            "wo": np.ascontiguousarray(W_out[cs:cs + 128, :]),
        }
        in_maps.append(m)
    return in_maps


def assemble(results, b_out):
    y = np.empty((B, N, D), dtype=np.float32)
    for bg in range(2):
        acc = results[bg * 4]["y"].copy()
        for hg in range(1, 4):
            acc += results[bg * 4 + hg]["y"]
        acc += np.asarray(b_out, dtype=np.float32)[None, :]
        y[2 * bg:2 * bg + 2] = acc.reshape(NB, N, D)
    return y


def kernel(x, W_qkv, W_out, b_out, f):
    assert int(f) == NF
    nc = _get_nc()
    in_maps = make_in_maps(x, W_qkv, W_out, b_out)
    res = bass_utils.run_bass_kernel_spmd(nc, in_maps, core_ids=list(range(8)))
    return assemble(res.results, b_out)


# revision 16
# speedup vs baseline: 1.4028x; 1.0354x over previous
"""TimeSformer divided-space attention (nn_Attention_3856880632239) on 8 TRN2 cores.

Sharding: core c = (batch-group bg = c//4) x (head-pair hg = c%4).
Each core handles 2 batches x 2 heads:
  - qkv projection in transposed layout (feature-on-partition), fp32r matmuls
  - per (b,h,frame): scoresT = k @ q^T (keys on partitions), exp (bf16),
    AV in natural orientation (queries on partitions) so softmax
    normalization is a per-partition scalar multiply, then PE-transpose back.
  - CLS query handled via query replication per frame section + partial-sum
    accumulation; CLS key handled via a batched rank-1 (K=1) matmul.
  - out projection with W_out row slice; partials summed on host.
"""
import numpy as np
import ml_dtypes
from contextlib import ExitStack

import concourse.bass as bass
from concourse import bacc
import concourse.tile as tile
import concourse.mybir as mybir
from concourse import bass_utils
from concourse.masks import make_identity

F32 = mybir.dt.float32
F32R = mybir.dt.float32r
BF16 = mybir.dt.bfloat16
AF = mybir.ActivationFunctionType
ALU = mybir.AluOpType

B, N, D, H, DH, NF, NP = 4, 3137, 512, 8, 64, 16, 196
NB = 2            # batches per core
SEC = NP + 1      # 197: section = [cls | frame queries]
QCOLS = 3584      # sectioned query buffer (16*197=3152, padded to 7*512)
KCOLS = N         # global key/row space
SCALE = DH ** -0.5


def _sec_pieces(g0, g1):
    """Global row range [g0,g1) (g0>=1) -> (src_off, dst_col, len) pieces."""
    out = []
    g = g0
    while g < g1:
        s, j = (g - 1) // NP, (g - 1) % NP
        run = min(g1 - g, NP - j)
        out.append((g - g0, SEC * s + 1 + j, run))
        g += run
    return out


def build():
    nc = bacc.Bacc("TRN2", target_bir_lowering=False, debug=False, num_devices=8)
    x = nc.dram_tensor("x", [NB * N, D], BF16, kind="ExternalInput")
    wq = nc.dram_tensor("wq", [D, 128], F32, kind="ExternalInput")
    wk = nc.dram_tensor("wk", [D, 128], F32, kind="ExternalInput")
    wv = nc.dram_tensor("wv", [D, 128], F32, kind="ExternalInput")
    wo = nc.dram_tensor("wo", [128, D], F32, kind="ExternalInput")
    y = nc.dram_tensor("y", [NB * N, D], F32, kind="ExternalOutput")

    with tile.TileContext(nc) as tc, ExitStack() as ctx:
        consts = ctx.enter_context(tc.tile_pool(name="consts", bufs=1))
        wrk = ctx.enter_context(tc.tile_pool(name="wrk", bufs=3))
        xtp = ctx.enter_context(tc.tile_pool(name="xtp", bufs=2))
        smallp = ctx.enter_context(tc.tile_pool(name="smallp", bufs=2))
        ps_big = ctx.enter_context(tc.tile_pool(name="ps_big", bufs=4, space="PSUM"))
        ps_avt = ctx.enter_context(tc.tile_pool(name="ps_avt", bufs=2, space="PSUM"))
        ps_at = ctx.enter_context(tc.tile_pool(name="ps_at", bufs=2, space="PSUM"))

        ident_f = consts.tile([128, 128], F32, tag="identf")
        make_identity(nc, ident_f[:])
        ident_b = consts.tile([128, 128], BF16, tag="identb")
        make_identity(nc, ident_b[:])

        # weights -> SBUF, rounded to f32r
        wq_f = consts.tile([128, 4, 128], F32, tag="wqf")
        wk_f = consts.tile([128, 4, 128], F32, tag="wkf")
        wv_f = consts.tile([128, 4, 128], F32, tag="wvf")
        for wt, wdr in ((wq_f, wq), (wk_f, wk), (wv_f, wv)):
            nc.sync.dma_start(wt[:], wdr[:].rearrange("(k p) c -> p k c", p=128))
        wq_r = consts.tile([128, 4, 128], BF16, tag="wqr")
        wk_r = consts.tile([128, 4, 128], BF16, tag="wkr")
        wv_r = consts.tile([128, 4, 128], BF16, tag="wvr")
        nc.scalar.copy(wq_r[:], wq_f[:])
        nc.scalar.copy(wk_r[:], wk_f[:])
        nc.scalar.copy(wv_r[:], wv_f[:])
        wo_f = consts.tile([128, D], F32, tag="wof")
        nc.sync.dma_start(wo_f[:], wo[:])
        wo_r = consts.tile([128, D], F32R, tag="wor")
        nc.scalar.copy(wo_r[:], wo_f[:])

        qT, kT, vT, aT = [], [], [], []
        for b in range(NB):
            qT.append(consts.tile([128, QCOLS], F32R, tag=f"qT{b}", name=f"qT{b}"))
            kT.append(consts.tile([128, QCOLS], F32R, tag=f"kT{b}", name=f"kT{b}"))
            vT.append(consts.tile([128, QCOLS], BF16, tag=f"vT{b}", name=f"vT{b}"))
            aT.append(consts.tile([128, KCOLS], F32R, tag=f"aT{b}", name=f"aT{b}"))
        zpad = consts.tile([128, QCOLS - 16 * SEC], F32, tag="zpad")
        nc.vector.memset(zpad[:], 0.0)
        for b in range(NB):
            nc.scalar.copy(qT[b][:, 16 * SEC:QCOLS], zpad[:])

        copy_flip = [0]

        def copy_alt(dst, src):
            copy_flip[0] ^= 1
            if copy_flip[0]:
                nc.scalar.copy(dst, src)
            else:
                nc.vector.tensor_copy(dst, src)

        # ---------------- Phase A: transpose x + qkv projection ------------
        def a_sb(b, r0):
                nsb = min(512, N - r0)
                xtt = xtp.tile([128, 4, 512], BF16, tag="xT")
                xa = wrk.tile([128, 4, 512], BF16, tag="xa")
                nfull = nsb // 128
                if nfull:
                    nc.sync.dma_start(
                        xa[:, 0:nfull, :],
                        x[b * N + r0: b * N + r0 + nfull * 128, :]
                        .rearrange("(c p) d -> p c d", p=128))
                if nsb % 128:
                    nc.sync.dma_start(
                        xa[0:nsb % 128, nfull, :],
                        x[b * N + r0 + nfull * 128: b * N + r0 + nsb, :])
                for rr0 in range(r0, r0 + nsb, 128):
                    nr = min(128, N - rr0)
                    c = (rr0 - r0) // 128
                    pack = ps_big.tile([128, 512], BF16, tag="big")
                    for k in range(4):
                        nc.tensor.transpose(
                            pack[:, k * 128: k * 128 + nr],
                            xa[0:nr, c, k * 128:(k + 1) * 128],
                            ident_b[0:nr, 0:nr])
                    off = rr0 - r0
                    copy_alt(
                        xtt[:, :, off:off + nr],
                        pack[:].rearrange("p (k c) -> p k c", k=4)[:, :, 0:nr])
                nmm = nsb + (nsb & 1)  # fp32r needs an even moving dim
                for wt, g in ((wq_r, 0), (wk_r, 1), (wv_r, 2)):
                    pp = ps_big.tile([128, 512], F32, tag="big")
                    for k in range(4):
                        nc.tensor.matmul(pp[:, 0:nmm], wt[:, k, :],
                                         xtt[:, k, 0:nmm],
                                         start=(k == 0), stop=(k == 3))
                    dst = (qT, kT, vT)[g]
                    cpy = nc.scalar.copy if g == 0 else nc.vector.tensor_copy
                    if r0 == 0:
                        for si in range(NF):
                            cpy(dst[b][:, SEC * si: SEC * si + 1], pp[:, 0:1])
                    g0 = max(r0, 1)
                    for src, dstc, ln in _sec_pieces(g0, r0 + nsb):
                        cpy(dst[b][:, dstc:dstc + ln],
                            pp[:, (g0 - r0) + src:(g0 - r0) + src + ln])

        def phase_a(b):
            for r0 in range(0, N, 512):
                a_sb(b, r0)

        # ---------------- Phase S: batched CLS-key scores ------------------
        # ---------------- Phase B: per-frame attention ---------------------
        def ve_prebuild(b, h):
            pb = h * 64
            veb = wrk.tile([128, NF * 130], BF16, tag="veb", name="veb", bufs=2)
            for f in range(NF):
                w0 = SEC * f
                vp = ps_avt.tile([128, 128], BF16, tag="avt", name="vp")
                nc.tensor.transpose(vp[0:128, 0:64], vT[b][pb:pb + 64, w0:w0 + 128],
                                    ident_b[pb:pb + 64, pb:pb + 64])
                nc.tensor.transpose(vp[0:69, 64:128],
                                    vT[b][pb:pb + 64, w0 + 128:w0 + 197],
                                    ident_b[pb:pb + 64, pb:pb + 64])
                o = 130 * f
                nc.vector.tensor_copy(
                    veb[0:128, o:o + 130].rearrange(
                        "p (g c) -> p g c", g=2)[:, :, 0:64],
                    vp[0:128, :].rearrange("p (g c) -> p g c", g=2))
            nc.gpsimd.memset(
                veb[0:128, :].rearrange("p (s c) -> p s c", s=NF)[:, :, 64:65], 1.0)
            nc.gpsimd.memset(
                veb[0:69, :].rearrange("p (s c) -> p s c", s=NF)[:, :, 129:130], 1.0)
            return veb

        def b_stage1(b, h, f, st):
            pb = h * 64
            w0 = SEC * f
            # scoresT: [keys, window-queries]; sectioned layout incl cls key
            sc = ps_big.tile([128, 512], F32, tag="big", name="sc")
            nc.tensor.matmul(sc[0:128, 0:256], kT[b][pb:pb + 64, w0:w0 + 128],
                             qT[b][pb:pb + 64, w0:w0 + 256], start=True, stop=True)
            nc.tensor.matmul(sc[0:69, 256:512], kT[b][pb:pb + 64, w0 + 128:w0 + 197],
                             qT[b][pb:pb + 64, w0:w0 + 256], start=True, stop=True)
            expT = wrk.tile([128, 394], BF16, tag="expT", name="expT", bufs=5)
            nc.scalar.activation(expT[0:128, 0:197], sc[0:128, 0:197],
                                 AF.Exp, scale=SCALE)
            nc.scalar.activation(expT[0:69, 197:394], sc[0:69, 256:453],
                                 AF.Exp, scale=SCALE)
            st["expT"] = expT

        def b_stage2(b, h, f, st):
            w0 = SEC * f
            clsacc = st["clsacc"]
            expT, veb = st["expT"], st["veb"]
            ve = veb[:, 130 * f:130 * f + 130]
            avp = ps_avt.tile([128, 130], F32, tag="avt", name="avp")
            nc.tensor.matmul(avp[0:128, 0:65], expT[0:128, 0:128], ve[0:128, 0:65],
                             start=True, stop=False)
            nc.tensor.matmul(avp[0:128, 0:65], expT[0:69, 197:325], ve[0:69, 65:130],
                             start=False, stop=True)
            nc.tensor.matmul(avp[0:69, 65:130], expT[0:128, 128:197], ve[0:128, 0:65],
                             start=True, stop=False)
            nc.tensor.matmul(avp[0:69, 65:130], expT[0:69, 325:394], ve[0:69, 65:130],
                             start=False, stop=True)
            # cls partial accumulation (row 0 = cls query)
            nc.vector.tensor_copy(clsacc[0:1, 65 * f:65 * f + 65], avp[0:1, 0:65])
            # normalize
            rinv = smallp.tile([128, 2], F32, tag="rinv")
            nc.vector.reciprocal(
                rinv[:], avp[0:128, :].rearrange("p (g c) -> p g c", g=2)[:, :, 64])
            ana = wrk.tile([128, 64], BF16, tag="ana", bufs=6)
            anb = wrk.tile([69, 64], BF16, tag="anb", bufs=6)
            nc.scalar.mul(ana[:], avp[0:128, 0:64], rinv[:, 0:1])
            nc.vector.tensor_scalar_mul(anb[:], avp[0:69, 65:129], rinv[0:69, 1:2])
            st["an"] = (ana, anb)

        def b_stage2b(b, h, f, st):
            pb = h * 64
            kc = 1 + NP * f
            ana, anb = st.pop("an_" + str(f))
            atp = ps_at.tile([64, 197], BF16, tag="at")
            nc.tensor.transpose(atp[0:64, 0:128], ana[:], ident_b[:])
            nc.tensor.transpose(atp[0:64, 128:197], anb[:], ident_b[0:69, 0:69])
            nc.vector.tensor_copy(aT[b][pb:pb + 64, kc:kc + NP], atp[0:64, 1:197])

        def cls_final(b, h, expT, veb, clsacc):
            pb = h * 64
            clssum = smallp.tile([1, 65], F32, tag="clssum")
            nc.vector.reduce_sum(
                clssum[:], clsacc[0:1, :].rearrange("p (s c) -> p c s", s=NF),
                axis=mybir.AxisListType.X)
            corr = smallp.tile([1, 65], F32, tag="corr")
            eself = smallp.tile([1, 1], F32, tag="eself")
            nc.vector.tensor_copy(eself[:], expT[0:1, 0:1])
            nc.vector.tensor_scalar(corr[:], veb[0:1, 0:65], eself[:],
                                    float(NF - 1), op0=ALU.mult, op1=ALU.mult)
            nc.vector.tensor_tensor(out=clssum[:], in0=clssum[:], in1=corr[:],
                                    op=ALU.subtract)
            crinv = smallp.tile([1, 1], F32, tag="crinv")
            nc.vector.reciprocal(crinv[:], clssum[0:1, 64:65])
            clsa = smallp.tile([1, 64], F32, tag="clsa")
            nc.vector.tensor_scalar_mul(clsa[:], clssum[0:1, 0:64], crinv[:])
            ctp = ps_avt.tile([64, 197], F32, tag="avt")
            nc.tensor.transpose(ctp[0:64, 0:1], clsa[:], ident_f[0:1, 0:1])
            nc.vector.tensor_copy(aT[b][pb:pb + 64, 0:1], ctp[0:64, 0:1])

        # ---------------- Phase C: out projection --------------------------
        def c_sb(b, s0):
                nsb = min(512, N - s0)
                ysb = wrk.tile([128, 4, 512], F32, tag="ysb")
                for r0 in range(s0, s0 + nsb, 128):
                    nr = min(128, N - r0)
                    c = (r0 - s0) // 128
                    yp = ps_big.tile([128, 512], F32, tag="big")
                    nc.tensor.matmul(yp[0:nr, :], aT[b][:, r0:r0 + nr], wo_r[:],
                                     start=True, stop=True)
                    copy_alt(ysb[0:nr, c, :], yp[0:nr, :])
                nfull = nsb // 128
                if nfull:
                    nc.sync.dma_start(
                        y[b * N + s0: b * N + s0 + nfull * 128, :]
                        .rearrange("(c p) d -> p c d", p=128),
                        ysb[:, 0:nfull, :])
                if nsb % 128:
                    nc.sync.dma_start(
                        y[b * N + s0 + nfull * 128: b * N + s0 + nsb, :],
                        ysb[0:nsb % 128, nfull, :])

        def b_units(b):
            units = []
            states = [{}, {}]

            def setup(h, state):
                clsacc = smallp.tile([1, 65 * NF], F32, tag="clsacc",
                                     name="clsacc")
                state.update(clsacc=clsacc, veb=ve_prebuild(b, h))
            units.append(lambda: setup(0, states[0]))
            units.append(lambda: setup(1, states[1]))
            # 3-deep software pipeline: s1(f) | s2a(f-1) | s2b(f-2)
            exp_hist = [{}, {}]

            def s1(h, f):
                b_stage1(b, h, f, states[h])
                exp_hist[h][f] = states[h]["expT"]

            def s2a(h, f):
                states[h]["expT"] = exp_hist[h].pop(f)
                b_stage2(b, h, f, states[h])
                states[h]["an_" + str(f)] = states[h].pop("an")

            def s2b(h, f):
                b_stage2b(b, h, f, states[h])

            for f in range(NF + 2):
                if f < NF:
                    units.append(lambda f=f: s1(0, f))
                    units.append(lambda f=f: s1(1, f))
                if 1 <= f <= NF:
                    units.append(lambda f=f: s2a(0, f - 1))
                    units.append(lambda f=f: s2a(1, f - 1))
                if f >= 2:
                    units.append(lambda f=f: s2b(0, f - 2))
                    units.append(lambda f=f: s2b(1, f - 2))
            units.append(lambda: cls_final(b, 0, states[0]["expT"],
                                           states[0]["veb"], states[0]["clsacc"]))
            units.append(lambda: cls_final(b, 1, states[1]["expT"],
                                           states[1]["veb"], states[1]["clsacc"]))
            return units

        def interleave(main, aux, period):
            ai = 0
            for i, u in enumerate(main):
                u()
                if (i + 1) % period == 0 and ai < len(aux):
                    aux[ai]()
                    ai += 1
            while ai < len(aux):
                aux[ai]()
                ai += 1

        a_units_1 = [lambda r0=r0: a_sb(1, r0) for r0 in range(0, N, 512)]
        c_units_0 = [lambda s0=s0: c_sb(0, s0) for s0 in range(0, N, 512)]
        c_units_1 = [lambda s0=s0: c_sb(1, s0) for s0 in range(0, N, 512)]

        phase_a(0)
        interleave(b_units(0), a_units_1, 9)
        interleave(b_units(1), c_units_0, 9)
        for u in c_units_1:
            u()
    nc.finalize()
    return nc


_NC_CACHE = None


def _get_nc():
    global _NC_CACHE
    if _NC_CACHE is None:
        _NC_CACHE = build()
    return _NC_CACHE


def make_in_maps(x, W_qkv, W_out, b_out):
    x = np.asarray(x, dtype=np.float32)
    W_qkv = np.asarray(W_qkv, dtype=np.float32)
    W_out = np.asarray(W_out, dtype=np.float32)
    b_out = np.asarray(b_out, dtype=np.float32)
    in_maps = []
    for c in range(8):
        bg, hg = c // 4, c % 4
        cs = hg * 128
        m = {
            "x": np.ascontiguousarray(
                x[2 * bg:2 * bg + 2].reshape(NB * N, D)).astype(
                    ml_dtypes.bfloat16),
            "wq": np.ascontiguousarray(W_qkv[:, cs:cs + 128]),
            "wk": np.ascontiguousarray(W_qkv[:, 512 + cs:512 + cs + 128]),
            "wv": np.ascontiguousarray(W_qkv[:, 1024 + cs:1024 + cs + 128]),
            "wo": np.ascontiguousarray(W_out[cs:cs + 128, :]),
        }
        in_maps.append(m)
    return in_maps


def assemble(results, b_out):
    y = np.empty((B, N, D), dtype=np.float32)
    for bg in range(2):
        acc = results[bg * 4]["y"].copy()
        for hg in range(1, 4):
            acc += results[bg * 4 + hg]["y"]
        acc += np.asarray(b_out, dtype=np.float32)[None, :]
        y[2 * bg:2 * bg + 2] = acc.reshape(NB, N, D)
    return y


def kernel(x, W_qkv, W_out, b_out, f):
    assert int(f) == NF
    nc = _get_nc()
    in_maps = make_in_maps(x, W_qkv, W_out, b_out)
    res = bass_utils.run_bass_kernel_spmd(nc, in_maps, core_ids=list(range(8)))
    return assemble(res.results, b_out)
